# revision 68
# baseline (speedup 1.0000x reference)
"""EpisodicMemory kernel for Trainium2, 8-core data-parallel. v2.

Reference computation (per batch b, d=32, m=64 memory slots, 2 hops):
    M = vs[b]
    for hop:
        Rh[m,:] = R[b,hop,m] @ h[b,hop,m]                  # batched matvec
        z = [Rh*v, Rh*M, |Rh-v|, |Rh-M|]                   # [m, 4d]
        Z = tanh(z @ W1.T + b1) @ W2.T (+ b2: dropped — softmax-invariant)
        g = softmax(Z over m); o = sum_m ts[b,hop,m] * g[m]
        M = GRUCell(o, M)
    out[b] = M

Sharding: pure data parallel over batch; 128 batches per core.

v2 design vs v1 (396 us -> 236 us), all choices HW-measured:
  - Einsum partition layout p=(q32, e4): 32 batches x 4 e-values.  R is
    host-permuted to [hop, S, eg, (q,e4), (d,m)] bf16 (dense 512 KB DMAs
    reach ~385 GB/s; fp8 was tried and rejected -- 1-byte operands drop
    DVE to the slow path, costing more than the DMA it saves).
  - P = R*h is ONE DVE broadcast-mul per e-group tile (h stride-0 over d,
    packed inner m: ~750 ns measured -- faster than packed h_rep
    materialization, contrary to the v1 session's note; Pool is 5x
    slower and gets none of the muls).
  - e-reduction on the TensorEngine: a quad of e-group P-tiles is first
    summed on DVE (3 adds, ~800 ns each; each add kills 4 PE matmuls),
    then contracted with a static block-diag-ones lhsT [128, 64]
    accumulating over 2 quads into [128, 512] PSUM chunks.  Super-blocks
    write 64-row halves (PSUM matmul outputs can only start at partition
    0/32/64).  Rh lands as [b=128, (m16, d32)] chunks, feature-ready.
  - Features: f0/f1 muls on DVE, f2/f3 subs on Pool (strided 512-f ops
    measured ~808 ns on Pool vs 1288 on DVE), abs on Act.
  - MLP: PE transpose z per m to z^T [feat*d, (m4, b)], W1 matmul + tanh
    into a [128, 512] a1 stack (32-aligned partition offsets), then one
    block-diag W2 matmul emits Z for 4 m-groups at once; Z returns to
    [b, m] via tiny PE transposes -- no DRAM bounce anywhere (the v1/v2.0
    Z gather DMA was descriptor-pathological: 4-byte partition stride).
  - Softmax skips the max-subtract (|Z| tanh-bounded), o is normalized
    after the t-reduction; GRU in transposed [d, b] layout with
    scalar_tensor_tensor-fused bias adds.
  - v_rep / M_rep are [b, (m16, d)] tiles built with a few packed SBUF
    copies -- the v1 DRAM broadcast bounce is gone.
"""

import numpy as np

import concourse.bacc as bacc
import concourse.bass as bass
import concourse.mybir as mybir
import concourse.tile as tile
from concourse.masks import make_identity

F32 = mybir.dt.float32
BF16 = mybir.dt.bfloat16
FP8 = mybir.dt.float8e4
AF = mybir.ActivationFunctionType
ALU = mybir.AluOpType
AX = mybir.AxisListType

B, N_HOP, N_MEM, DIM = 1024, 2, 64, 32
N_CORES = 8
BC = B // N_CORES            # 128 batches per core
NSB = 4                      # super-blocks per core
QB = 32                      # batches per super-block (partition q-dim)
EV = 4                       # e-values per partition group
NEG = DIM // EV              # 8 e-groups
MC = 16                      # m per PE output chunk
NCH = N_MEM // MC            # 4 chunks
D4 = 4 * DIM                 # 128 MLP input features




def build_nc(n_iter: int = 1, variant: str = "full") -> bass.Bass:
    """variant: 'full' | 'dma' (loads only)"""
    nc = bacc.Bacc("TRN2")

    # host-permuted layouts (b = S*32 + q within a core):
    #   Rs[hop, S, eg, (q,e4), (d,m)]   bf16
    #   hs[hop, S, (q,e4), (eg,m)]      bf16
    Rs_d = nc.dram_tensor(
        "Rs", [N_HOP, NSB, NEG, 128, DIM * N_MEM], BF16, kind="ExternalInput"
    )
    hs_d = nc.dram_tensor(
        "hs", [N_HOP, NSB, 128, NEG * N_MEM], BF16, kind="ExternalInput"
    )
    ts_d = nc.dram_tensor("ts", [BC, N_HOP, N_MEM, DIM], BF16, kind="ExternalInput")
    vs_d = nc.dram_tensor("vs", [BC, DIM], F32, kind="ExternalInput")
    W1_d = nc.dram_tensor("W1", [DIM, D4], F32, kind="ExternalInput")
    b1_d = nc.dram_tensor("b1", [DIM], F32, kind="ExternalInput")
    W2_d = nc.dram_tensor("W2", [1, DIM], F32, kind="ExternalInput")
    Wih_d = nc.dram_tensor("W_ih", [N_HOP, 3 * DIM, DIM], F32, kind="ExternalInput")
    Whh_d = nc.dram_tensor("W_hh", [N_HOP, 3 * DIM, DIM], F32, kind="ExternalInput")
    bih_d = nc.dram_tensor("b_ih", [N_HOP, 3 * DIM], F32, kind="ExternalInput")
    bhh_d = nc.dram_tensor("b_hh", [N_HOP, 3 * DIM], F32, kind="ExternalInput")
    out_d = nc.dram_tensor("out", [BC, DIM], F32, kind="ExternalOutput")

    import contextlib

    with tile.TileContext(nc) as tc:
        with (
            (tc.For_i(0, n_iter, 1) if n_iter > 1 else contextlib.nullcontext()),
            tc.tile_pool(name="consts", bufs=1) as consts,
            tc.tile_pool(name="hop_io", bufs=2) as hop_io,
            tc.tile_pool(name="rpool", bufs=12) as rpool,
            tc.tile_pool(name="ppool", bufs=2) as ppool,
            tc.tile_pool(name="fpool", bufs=2) as fpool,
            tc.tile_pool(name="zpool", bufs=2) as zpool,
            tc.tile_pool(name="apool", bufs=2) as apool,
            tc.tile_pool(name="small", bufs=2) as small,
            tc.tile_pool(name="mstate", bufs=2) as mstate,
            tc.tile_pool(name="pp_rh", bufs=1, space="PSUM") as pp_rh,
            tc.tile_pool(name="pp_zt", bufs=1, space="PSUM") as pp_zt,
            tc.tile_pool(name="pp_m", bufs=2, space="PSUM") as pp_m,
            tc.tile_pool(name="pp_2", bufs=1, space="PSUM") as pp_2,
        ):
            ident = consts.tile([128, 128], F32)
            make_identity(nc, ident)
            ident16 = consts.tile([128, 128], BF16)
            nc.scalar.copy(out=ident16, in_=ident)

            # block-diag ones [p=(q,e4), q]: 1 iff p//4 == q.  PSUM matmul
            # outputs may only start at partition 0/32/64, so super-blocks
            # write 64-row halves: lo has the diag in cols 0-31 (S even),
            # hi in cols 32-63 (S odd); the other half-block's rows get +0.
            a2 = ident.rearrange("p (q two) -> p q two", two=2)
            t64 = consts.tile([128, 64], F32)
            nc.vector.tensor_add(t64, a2[:, :, 0], a2[:, :, 1])
            b2v = t64.rearrange("p (q two) -> p q two", two=2)
            t32 = consts.tile([128, 32], F32)
            nc.vector.tensor_add(t32, b2v[:, :, 0], b2v[:, :, 1])
            ones_lo = consts.tile([128, 64], BF16)
            nc.vector.memset(ones_lo, 0)
            nc.scalar.copy(out=ones_lo[:, 0:32], in_=t32)
            ones_hi = consts.tile([128, 64], BF16)
            nc.vector.memset(ones_hi, 0)
            nc.scalar.copy(out=ones_hi[:, 32:64], in_=t32)

            # ---- weights prep (one-time) ----
            w1_sb = consts.tile([DIM, D4], F32)
            nc.sync.dma_start(out=w1_sb, in_=W1_d[:, :])
            w1t_ps = pp_m.tile([D4, DIM], F32, tag="ps1")
            nc.tensor.transpose(w1t_ps, w1_sb, ident[:DIM, :DIM])
            W1T = consts.tile([D4, DIM], BF16)
            nc.scalar.copy(out=W1T, in_=w1t_ps)

            W2T_f = consts.tile([DIM, 1], F32)
            nc.sync.dma_start(out=W2T_f, in_=W2_d.rearrange("a b -> b a"))
            # block-diag W2^T [128, 4]: col j = W2^T at partitions 32j..
            w2bd_f = consts.tile([128, EV], F32)
            nc.vector.memset(w2bd_f, 0)
            for j in range(EV):
                nc.scalar.copy(
                    out=w2bd_f[j * DIM : (j + 1) * DIM, j : j + 1], in_=W2T_f
                )
            W2BD = consts.tile([128, EV], BF16)
            nc.scalar.copy(out=W2BD, in_=w2bd_f)
            b1T = consts.tile([DIM, 1], F32)
            nc.sync.dma_start(out=b1T, in_=b1_d[:].unsqueeze(1))
            b1T2 = consts.tile([2 * DIM, 1], F32)
            nc.scalar.copy(out=b1T2[0:DIM, :], in_=b1T)
            nc.scalar.copy(out=b1T2[DIM : 2 * DIM, :], in_=b1T)

            WihT, WhhT, bsum_rz, bihn_t, bhhn_t = [], [], [], [], []
            for hop in range(N_HOP):
                wih_sb = consts.tile([3 * DIM, DIM], F32, tag="wload", bufs=4)
                nc.sync.dma_start(out=wih_sb, in_=Wih_d[hop])
                wt_ps = pp_m.tile([DIM, 3 * DIM], F32, tag="ps1")
                nc.tensor.transpose(wt_ps, wih_sb, ident[: 3 * DIM, : 3 * DIM])
                wT = consts.tile([DIM, 3 * DIM], F32, tag=f"wihT{hop}")
                nc.scalar.copy(out=wT, in_=wt_ps)
                WihT.append(wT)

                whh_sb = consts.tile([3 * DIM, DIM], F32, tag="wload", bufs=4)
                nc.sync.dma_start(out=whh_sb, in_=Whh_d[hop])
                wt_ps2 = pp_m.tile([DIM, 3 * DIM], F32, tag="ps1")
                nc.tensor.transpose(wt_ps2, whh_sb, ident[: 3 * DIM, : 3 * DIM])
                wT2 = consts.tile([DIM, 3 * DIM], F32, tag=f"whhT{hop}")
                nc.scalar.copy(out=wT2, in_=wt_ps2)
                WhhT.append(wT2)

                gate_b = []
                for gd, gname in ((bih_d, "ih"), (bhh_d, "hh")):
                    for gate in range(3):
                        bt = consts.tile([DIM, 1], F32, tag=f"b{gname}{hop}{gate}")
                        nc.sync.dma_start(
                            out=bt,
                            in_=gd[hop, gate * DIM : (gate + 1) * DIM].unsqueeze(1),
                        )
                        gate_b.append(bt)
                b_r = consts.tile([DIM, 1], F32, tag=f"b_r{hop}")
                nc.vector.tensor_add(b_r, gate_b[0], gate_b[3])
                b_z = consts.tile([DIM, 1], F32, tag=f"b_z{hop}")
                nc.vector.tensor_add(b_z, gate_b[1], gate_b[4])
                bsum_rz.append((b_r, b_z))
                bihn_t.append(gate_b[2])
                bhhn_t.append(gate_b[5])

            # ---- initial M state ----
            vs_row = consts.tile([BC, DIM], F32)
            nc.sync.dma_start(out=vs_row, in_=vs_d[:, :])
            vst_ps = pp_m.tile([DIM, BC], F32, tag="ps1")
            nc.tensor.transpose(vst_ps, vs_row, ident)
            vsT = consts.tile([DIM, BC], F32)
            nc.scalar.copy(out=vsT, in_=vst_ps)
            MT = vsT  # current M^T [d, b]

            # v_rep [b, (m16, d)] bf16 via packed log-doubling
            v_rep = consts.tile([BC, MC * DIM], BF16)
            nc.vector.tensor_copy(v_rep[:, 0:DIM], vs_row)
            w = DIM
            while w < MC * DIM:
                nc.vector.tensor_copy(v_rep[:, w : 2 * w], v_rep[:, 0:w])
                w *= 2

            M_rep = v_rep  # hop 0: M == vs

            for hop in range(N_HOP):
                # h for the whole hop+sblk: [p=(q,e4), (eg, m)]
                h_sb = []
                for S in range(NSB):
                    h_t = hop_io.tile([128, NEG * N_MEM], BF16, tag="h", bufs=4)
                    nc.scalar.dma_start(out=h_t, in_=hs_d[hop, S])
                    h_sb.append(h_t)
                t_hop = hop_io.tile([BC, N_MEM * DIM], BF16, tag="t_hop")
                nc.scalar.dma_start(
                    out=t_hop, in_=ts_d[:, hop].rearrange("b m d -> b (m d)")
                )

                # Rh accumulators: [b=128, (m16, d32)] f32, one per m-chunk
                rh_ps = [
                    pp_rh.tile([128, MC * DIM], F32, tag=f"rh{c}", name=f"rh{c}")
                    for c in range(NCH)
                ]

                # ---- einsum: Rh = sum_e R*h via PE block-diag reduce ----
                for S in range(NSB):
                    half = (S // 2) * 64
                    ones_bd = ones_lo if S % 2 == 0 else ones_hi
                    for quad in range(NEG // 4):
                        P_ts = []
                        A01 = A23 = None
                        for e4i in range(4):
                            eg = quad * 4 + e4i
                            r_t = rpool.tile(
                                [128, DIM * N_MEM], BF16, tag="R", bufs=16
                            )
                            # alternate HWDGE queues: one queue saturates at
                            # ~385 GB/s for these 512 KB tiles
                            r_eng = nc.sync if eg % 2 == 0 else nc.scalar
                            r_eng.dma_start(out=r_t, in_=Rs_d[hop, S, eg])
                            if variant == "dma":
                                continue
                            # single DVE broadcast-mul (h stride-0 over d,
                            # packed inner m): ~750 ns measured on HW
                            P_t = ppool.tile(
                                [128, DIM * N_MEM], BF16, tag="P", bufs=8
                            )
                            h_sl = h_sb[S][
                                :, eg * N_MEM : (eg + 1) * N_MEM
                            ]
                            h_b = h_sl.unsqueeze(1).broadcast_to(
                                (128, DIM, N_MEM)
                            )
                            nc.vector.tensor_tensor(
                                P_t.rearrange("p (d m) -> p d m", d=DIM),
                                r_t.rearrange("p (d m) -> p d m", d=DIM),
                                h_b,
                                op=ALU.mult,
                            )
                            P_ts.append(P_t)
                            if variant == "nomm":
                                continue
                            # quad-sum tree on DVE, emitted as operands land
                            if e4i == 1:
                                A01 = ppool.tile(
                                    [128, DIM * N_MEM], BF16, tag="PA", bufs=3
                                )
                                nc.vector.tensor_add(A01, P_ts[0], P_ts[1])
                            elif e4i == 3:
                                A23 = ppool.tile(
                                    [128, DIM * N_MEM], BF16, tag="PB", bufs=3
                                )
                                nc.vector.tensor_add(A23, P_ts[2], P_ts[3])
                        if variant in ("dma", "nomm"):
                            continue
                        AQ = ppool.tile([128, DIM * N_MEM], BF16, tag="PQ",
                                        bufs=3)
                        nc.vector.tensor_add(AQ, A01, A23)
                        Pm = AQ.rearrange("p (d m) -> p m d", d=DIM)
                        for c in range(NCH):
                            nc.tensor.matmul(
                                rh_ps[c][half : half + 64, :],
                                lhsT=ones_bd,
                                rhs=Pm[:, c * MC : (c + 1) * MC, :],
                                start=(S % 2 == 0 and quad == 0),
                                stop=(S % 2 == 1 and quad == NEG // 4 - 1),
                            )

                if variant in ("dma", "nomm"):
                    continue
                if variant == "notail":
                    # consume rh into out cheaply to keep deps
                    if hop == N_HOP - 1:
                        M_row = mstate.tile([BC, DIM], F32, tag="M_row")
                        nc.scalar.copy(out=M_row, in_=rh_ps[0][:, 0:DIM])
                        nc.sync.dma_start(out=out_d[:, :], in_=M_row)
                    else:
                        for c in range(NCH):
                            rh_sb = fpool.tile([BC, MC * DIM], BF16, tag="rh_sb")
                            nc.scalar.copy(out=rh_sb, in_=rh_ps[c])
                    continue

                # ---- features z = [Rh*v, Rh*M, |Rh-v|, |Rh-M|] ----
                z_hop = zpool.tile([BC, N_MEM * 4 * DIM], BF16, tag="z")
                z4 = z_hop.rearrange("b (m f d) -> b m f d", f=4, d=DIM)
                vr3 = v_rep.rearrange("b (m d) -> b m d", d=DIM)
                mr3 = M_rep.rearrange("b (m d) -> b m d", d=DIM)
                for c in range(NCH):
                    mc = slice(c * MC, (c + 1) * MC)
                    rh_sb = fpool.tile([BC, MC * DIM], BF16, tag="rh_sb")
                    if c % 2 == 0:
                        nc.scalar.copy(out=rh_sb, in_=rh_ps[c])
                    else:
                        nc.vector.tensor_copy(rh_sb, rh_ps[c])
                    rh3 = rh_sb.rearrange("b (m d) -> b m d", d=DIM)
                    nc.vector.tensor_mul(z4[:, mc, 0, :], rh3, vr3)
                    nc.vector.tensor_mul(z4[:, mc, 1, :], rh3, mr3)
                    nc.gpsimd.tensor_tensor(
                        z4[:, mc, 2, :], rh3, vr3, op=ALU.subtract
                    )
                    nc.gpsimd.tensor_tensor(
                        z4[:, mc, 3, :], rh3, mr3, op=ALU.subtract
                    )
                    nc.scalar.activation(
                        z4[:, mc, 2:4, :], z4[:, mc, 2:4, :], AF.Abs
                    )

                # ---- MLP per m4-group: transpose + matmuls; groups of 4
                # stack a1 into [128, 512] (32-aligned partition offsets) so
                # one block-diag W2 matmul emits Z for 4 groups at once.
                # Final Z col-order is m' = (m4, G, g4); ts is host-permuted
                # to match (softmax is order-invariant).
                zT_ps = pp_m.tile([BC, N_MEM], F32, tag="ps1")
                zf = z_hop.rearrange("b (m fd) -> b m fd", fd=4 * DIM)
                for G in range(EV):
                    a1_4 = apool.tile([128, EV * BC], BF16, tag="a1")
                    for g4 in range(EV):
                        g = G * EV + g4
                        zt_ps = pp_zt.tile([D4, EV * BC], BF16, tag="zt")
                        for j in range(EV):
                            nc.tensor.transpose(
                                zt_ps[:, j * BC : (j + 1) * BC],
                                zf[:, g * EV + j, :],
                                ident16,
                            )
                        zt_sb = zpool.tile(
                            [D4, EV * BC], BF16, tag="zt_sb", bufs=3
                        )
                        if g % 2 == 0:
                            nc.scalar.copy(out=zt_sb, in_=zt_ps)
                        else:
                            nc.vector.tensor_copy(zt_sb, zt_ps)
                        ps1 = pp_m.tile([DIM, EV * BC], F32, tag="ps1")
                        nc.tensor.matmul(
                            ps1, lhsT=W1T, rhs=zt_sb, start=True, stop=True
                        )
                        nc.scalar.activation(
                            a1_4[g4 * DIM : (g4 + 1) * DIM, :],
                            ps1,
                            AF.Tanh,
                            bias=b1T,
                        )
                    ps2 = pp_2.tile([EV, EV * BC], F32, tag="ps2")
                    nc.tensor.matmul(ps2, lhsT=W2BD, rhs=a1_4, start=True, stop=True)
                    z4sb = zpool.tile([EV, EV * BC], F32, tag="z4sb", bufs=2)
                    if G % 2 == 0:
                        nc.scalar.copy(out=z4sb, in_=ps2)
                    else:
                        nc.vector.tensor_copy(z4sb, ps2)
                    for j in range(EV):
                        nc.tensor.transpose(
                            zT_ps[:, j * MC + G * EV : j * MC + (G + 1) * EV],
                            z4sb[:, j * BC : (j + 1) * BC],
                            ident[:EV, :EV],
                        )

                # softmax over m (skip max-subtract; |Z| tanh-bounded),
                # normalize o after the t-reduction
                e_row = small.tile([BC, N_MEM], F32, tag="e_row")
                nc.scalar.activation(e_row, zT_ps, AF.Exp)
                e16 = small.tile([BC, N_MEM], BF16, tag="e16")
                nc.scalar.copy(out=e16, in_=e_row)
                ssum = small.tile([BC, 1], F32, tag="ssum")
                nc.vector.tensor_reduce(out=ssum, in_=e_row, axis=AX.X, op=ALU.add)
                rsum = small.tile([BC, 1], F32, tag="rsum")
                nc.vector.reciprocal(rsum, ssum)

                # o[b,d] = (sum_m t[b,m,d] * e[b,m]) * rsum[b]
                t3 = t_hop.rearrange("b (m d) -> b m d", d=DIM)
                g3 = e16.unsqueeze(2).broadcast_to((BC, N_MEM, DIM))
                nc.vector.tensor_tensor(t3, t3, g3, op=ALU.mult)
                for mh in (32, 16, 8, 4, 2):
                    nc.vector.tensor_add(
                        t3[:, :mh, :], t3[:, :mh, :], t3[:, mh : 2 * mh, :]
                    )
                o_raw = small.tile([BC, DIM], F32, tag="o_raw")
                nc.vector.tensor_add(
                    o_raw.unsqueeze(1), t3[:, 0:1, :], t3[:, 1:2, :]
                )
                o_row = small.tile([BC, DIM], F32, tag="o_row")
                nc.vector.tensor_scalar_mul(o_row, o_raw, rsum)

                # ---- GRU (transposed layout [*, b], f32) ----
                ot_ps = pp_m.tile([DIM, BC], F32, tag="ps1")
                nc.tensor.transpose(ot_ps, o_row, ident)
                oT = small.tile([DIM, BC], F32, tag="oT")
                nc.scalar.copy(out=oT, in_=ot_ps)

                def gate_pair(g):
                    gi = pp_m.tile([DIM, BC], F32, tag="ps1")
                    nc.tensor.matmul(
                        gi,
                        lhsT=WihT[hop][:, g * DIM : (g + 1) * DIM],
                        rhs=oT,
                        start=True,
                        stop=True,
                    )
                    gh = pp_m.tile([DIM, BC], F32, tag="ps1")
                    nc.tensor.matmul(
                        gh,
                        lhsT=WhhT[hop][:, g * DIM : (g + 1) * DIM],
                        rhs=MT,
                        start=True,
                        stop=True,
                    )
                    return gi, gh

                rz_t = []
                for g in range(2):
                    gi, gh = gate_pair(g)
                    gh_sb = small.tile([DIM, BC], F32, tag=f"gh{g}sb")
                    nc.vector.tensor_copy(gh_sb, gh)
                    gb = small.tile([DIM, BC], F32, tag=f"g{g}b")
                    # (gi + b) + gh in one DVE op (only one PSUM operand)
                    nc.vector.scalar_tensor_tensor(
                        out=gb, in0=gi, scalar=bsum_rz[hop][g], in1=gh_sb,
                        op0=ALU.add, op1=ALU.add,
                    )
                    gt = small.tile([DIM, BC], F32, tag=f"gate{g}")
                    nc.scalar.activation(gt, gb, AF.Sigmoid)
                    rz_t.append(gt)
                r_t, z_t = rz_t

                gi_n, gh_n = gate_pair(2)
                n1 = small.tile([DIM, BC], F32, tag="n1")
                # (gh_n + b_hh_n) * r
                nc.vector.scalar_tensor_tensor(
                    out=n1, in0=gh_n, scalar=bhhn_t[hop], in1=r_t,
                    op0=ALU.add, op1=ALU.mult,
                )
                n2 = small.tile([DIM, BC], F32, tag="n2")
                # (gi_n + b_ih_n) + n1
                nc.vector.scalar_tensor_tensor(
                    out=n2, in0=gi_n, scalar=bihn_t[hop], in1=n1,
                    op0=ALU.add, op1=ALU.add,
                )
                n_t = small.tile([DIM, BC], F32, tag="n_t")
                nc.scalar.activation(n_t, n2, AF.Tanh)

                # M' = n + z * (M - n)
                MT_new = mstate.tile([DIM, BC], F32, tag="MT")
                nc.vector.tensor_sub(MT_new, MT, n_t)
                nc.vector.tensor_mul(MT_new, MT_new, z_t)
                nc.vector.tensor_add(MT_new, MT_new, n_t)
                MT = MT_new

                mrow_ps = pp_m.tile([BC, DIM], F32, tag="ps1")
                nc.tensor.transpose(mrow_ps, MT, ident[:DIM, :DIM])
                if hop < N_HOP - 1:
                    # M_rep [b, (m16, d)] bf16 via packed log-doubling
                    M_rep_new = mstate.tile([BC, MC * DIM], BF16, tag="M_rep")
                    nc.scalar.copy(out=M_rep_new[:, 0:DIM], in_=mrow_ps)
                    w = DIM
                    while w < MC * DIM:
                        nc.vector.tensor_copy(
                            M_rep_new[:, w : 2 * w], M_rep_new[:, 0:w]
                        )
                        w *= 2
                    M_rep = M_rep_new
                else:
                    M_row = mstate.tile([BC, DIM], F32, tag="M_row")
                    nc.scalar.copy(out=M_row, in_=mrow_ps)
                    nc.sync.dma_start(out=out_d[:, :], in_=M_row)

    nc.compile()
    return nc


_NC_CACHE = None


def _get_nc():
    global _NC_CACHE
    if _NC_CACHE is None:
        _NC_CACHE = build_nc()
    return _NC_CACHE


def _bf16(x):
    import ml_dtypes

    return np.asarray(x).astype(ml_dtypes.bfloat16)


def _fp8(x):
    import ml_dtypes

    return np.asarray(x).astype(ml_dtypes.float8_e4m3)


def permute_R(x):
    """Rs [BC, N_HOP, m, d, e] -> [hop, S, eg, (q,e4), (d,m)] bf16."""
    y = x.reshape(NSB, QB, N_HOP, N_MEM, DIM, NEG, EV)
    # [S, q, hop, m, d, eg, e4] -> [hop, S, eg, q, e4, d, m]
    y = y.transpose(2, 0, 5, 1, 6, 4, 3).reshape(
        N_HOP, NSB, NEG, 128, DIM * N_MEM
    )
    return np.ascontiguousarray(_bf16(y))


def permute_h(x):
    """hs [BC, N_HOP, m, e] -> [hop, S, (q,e4), (eg,m)] bf16."""
    y = x.reshape(NSB, QB, N_HOP, N_MEM, NEG, EV)
    # [S, q, hop, m, eg, e4] -> [hop, S, q, e4, eg, m]
    y = y.transpose(2, 0, 1, 5, 4, 3)
    return np.ascontiguousarray(y.reshape(N_HOP, NSB, 128, NEG * N_MEM))


def permute_t(x):
    """ts [BC, hop, m, d]: m reordered to m' = (m4, G, g4), m = (4G+g4)*4+m4."""
    y = x.reshape(BC, N_HOP, EV, EV, EV, DIM)  # [b, hop, G, g4, m4, d]
    return np.ascontiguousarray(
        y.transpose(0, 1, 4, 2, 3, 5).reshape(BC, N_HOP, N_MEM, DIM)
    )


def make_in_maps(hs, Rs, ts, vs, W1, b1, W2, W_ih, W_hh, b_ih, b_hh):
    in_maps = []
    for c in range(N_CORES):
        sl = slice(c * BC, (c + 1) * BC)
        in_maps.append(
            {
                "Rs": permute_R(Rs[sl]),
                "hs": permute_h(_bf16(hs[sl])),
                "ts": permute_t(_bf16(ts[sl])),
                "vs": np.ascontiguousarray(vs[sl]),
                "W1": np.ascontiguousarray(W1),
                "b1": np.ascontiguousarray(b1),
                "W2": np.ascontiguousarray(W2),
                "W_ih": np.ascontiguousarray(W_ih),
                "W_hh": np.ascontiguousarray(W_hh),
                "b_ih": np.ascontiguousarray(b_ih),
                "b_hh": np.ascontiguousarray(b_hh),
            }
        )
    return in_maps


def kernel(hs, Rs, ts, vs, W1, b1, W2, b2, W_ih, W_hh, b_ih, b_hh):
    from concourse.bass_utils import run_bass_kernel_spmd

    nc = _get_nc()
    in_maps = make_in_maps(hs, Rs, ts, vs, W1, b1, W2, W_ih, W_hh, b_ih, b_hh)
    res = run_bass_kernel_spmd(nc, in_maps, list(range(N_CORES)))
    return np.concatenate([r["out"] for r in res.results], axis=0)


# revision 71
# speedup vs baseline: 1.1766x; 1.1766x over previous
"""EpisodicMemory kernel for Trainium2, 8-core data-parallel. v2.

Reference computation (per batch b, d=32, m=64 memory slots, 2 hops):
    M = vs[b]
    for hop:
        Rh[m,:] = R[b,hop,m] @ h[b,hop,m]                  # batched matvec
        z = [Rh*v, Rh*M, |Rh-v|, |Rh-M|]                   # [m, 4d]
        Z = tanh(z @ W1.T + b1) @ W2.T (+ b2: dropped — softmax-invariant)
        g = softmax(Z over m); o = sum_m ts[b,hop,m] * g[m]
        M = GRUCell(o, M)
    out[b] = M

Sharding: pure data parallel over batch; 128 batches per core.

v2 design vs v1 (396 us -> 236 us), all choices HW-measured:
  - Einsum partition layout p=(q32, e4): 32 batches x 4 e-values.  R is
    host-permuted to [hop, S, eg, (q,e4), (d,m)] bf16 (dense 512 KB DMAs
    reach ~385 GB/s; fp8 was tried and rejected -- 1-byte operands drop
    DVE to the slow path, costing more than the DMA it saves).
  - P = R*h is ONE DVE broadcast-mul per e-group tile (h stride-0 over d,
    packed inner m: ~750 ns measured -- faster than packed h_rep
    materialization, contrary to the v1 session's note; Pool is 5x
    slower and gets none of the muls).
  - e-reduction on the TensorEngine: a quad of e-group P-tiles is first
    summed on DVE (3 adds, ~800 ns each; each add kills 4 PE matmuls),
    then contracted with a static block-diag-ones lhsT [128, 64]
    accumulating over 2 quads into [128, 512] PSUM chunks.  Super-blocks
    write 64-row halves (PSUM matmul outputs can only start at partition
    0/32/64).  Rh lands as [b=128, (m16, d32)] chunks, feature-ready.
  - Features: f0/f1 muls on DVE, f2/f3 subs on Pool (strided 512-f ops
    measured ~808 ns on Pool vs 1288 on DVE), abs on Act.
  - MLP: PE transpose z per m to z^T [feat*d, (m4, b)], W1 matmul + tanh
    into a [128, 512] a1 stack (32-aligned partition offsets), then one
    block-diag W2 matmul emits Z for 4 m-groups at once; Z returns to
    [b, m] via tiny PE transposes -- no DRAM bounce anywhere (the v1/v2.0
    Z gather DMA was descriptor-pathological: 4-byte partition stride).
  - Softmax skips the max-subtract (|Z| tanh-bounded), o is normalized
    after the t-reduction; GRU in transposed [d, b] layout with
    scalar_tensor_tensor-fused bias adds.
  - v_rep / M_rep are [b, (m16, d)] tiles built with a few packed SBUF
    copies -- the v1 DRAM broadcast bounce is gone.

Measured dead ends (do not retry without new evidence): fp8 R loads
(DVE 1-byte slow path), Pool-heavy muls/features, packed h_rep, stacked
2-group tanh, max-form abs, all-DVE PSUM copies, emitting both hops'
einsums before the tails (delays the serial GRU chain), pre-add depths
1 and 3 (depth 2 is the optimum), R DMAs split onto the Act HWDGE queue
(transfers contend with Act's tail compute), deeper R/P buffering.
Untried with upside: a cheap fp8 P-producer to unlock PE DoubleRow
reduction; sub-hop software pipelining of einsum vs tail.
"""

import numpy as np

import concourse.bacc as bacc
import concourse.bass as bass
import concourse.mybir as mybir
import concourse.tile as tile
from concourse.masks import make_identity

F32 = mybir.dt.float32
BF16 = mybir.dt.bfloat16
FP8 = mybir.dt.float8e4
AF = mybir.ActivationFunctionType
ALU = mybir.AluOpType
AX = mybir.AxisListType

B, N_HOP, N_MEM, DIM = 1024, 2, 64, 32
N_CORES = 8
BC = B // N_CORES            # 128 batches per core
NSB = 4                      # super-blocks per core
QB = 32                      # batches per super-block (partition q-dim)
EV = 4                       # e-values per partition group
NEG = DIM // EV              # 8 e-groups
MC = 16                      # m per PE output chunk
NCH = N_MEM // MC            # 4 chunks
D4 = 4 * DIM                 # 128 MLP input features




def build_nc(n_iter: int = 1, variant: str = "full") -> bass.Bass:
    """variant: 'full' | 'dma' (loads only)"""
    nc = bacc.Bacc("TRN2")

    # host-permuted layouts (b = S*32 + q within a core):
    #   Rs[hop, S, eg, (q,e4), (d,m)]   bf16
    #   hs[hop, S, (q,e4), (eg,m)]      bf16
    Rs_d = nc.dram_tensor(
        "Rs", [N_HOP, NSB, NEG, 128, DIM * N_MEM], BF16, kind="ExternalInput"
    )
    hs_d = nc.dram_tensor(
        "hs", [N_HOP, NSB, 128, NEG * N_MEM], BF16, kind="ExternalInput"
    )
    ts_d = nc.dram_tensor("ts", [BC, N_HOP, N_MEM, DIM], BF16, kind="ExternalInput")
    vs_d = nc.dram_tensor("vs", [BC, DIM], F32, kind="ExternalInput")
    W1_d = nc.dram_tensor("W1", [DIM, D4], F32, kind="ExternalInput")
    b1_d = nc.dram_tensor("b1", [DIM], F32, kind="ExternalInput")
    W2_d = nc.dram_tensor("W2", [1, DIM], F32, kind="ExternalInput")
    Wih_d = nc.dram_tensor("W_ih", [N_HOP, 3 * DIM, DIM], F32, kind="ExternalInput")
    Whh_d = nc.dram_tensor("W_hh", [N_HOP, 3 * DIM, DIM], F32, kind="ExternalInput")
    bih_d = nc.dram_tensor("b_ih", [N_HOP, 3 * DIM], F32, kind="ExternalInput")
    bhh_d = nc.dram_tensor("b_hh", [N_HOP, 3 * DIM], F32, kind="ExternalInput")
    out_d = nc.dram_tensor("out", [BC, DIM], F32, kind="ExternalOutput")

    import contextlib

    with tile.TileContext(nc) as tc:
        with (
            (tc.For_i(0, n_iter, 1) if n_iter > 1 else contextlib.nullcontext()),
            tc.tile_pool(name="consts", bufs=1) as consts,
            tc.tile_pool(name="hop_io", bufs=2) as hop_io,
            tc.tile_pool(name="rpool", bufs=12) as rpool,
            tc.tile_pool(name="ppool", bufs=2) as ppool,
            tc.tile_pool(name="fpool", bufs=2) as fpool,
            tc.tile_pool(name="zpool", bufs=2) as zpool,
            tc.tile_pool(name="apool", bufs=2) as apool,
            tc.tile_pool(name="small", bufs=2) as small,
            tc.tile_pool(name="mstate", bufs=2) as mstate,
            tc.tile_pool(name="pp_rh", bufs=1, space="PSUM") as pp_rh,
            tc.tile_pool(name="pp_zt", bufs=1, space="PSUM") as pp_zt,
            tc.tile_pool(name="pp_m", bufs=2, space="PSUM") as pp_m,
            tc.tile_pool(name="pp_2", bufs=1, space="PSUM") as pp_2,
        ):
            ident = consts.tile([128, 128], F32)
            make_identity(nc, ident)
            ident16 = consts.tile([128, 128], BF16)
            nc.scalar.copy(out=ident16, in_=ident)

            # block-diag ones [p=(q,e4), q]: 1 iff p//4 == q.  PSUM matmul
            # outputs may only start at partition 0/32/64, so super-blocks
            # write 64-row halves: lo has the diag in cols 0-31 (S even),
            # hi in cols 32-63 (S odd); the other half-block's rows get +0.
            a2 = ident.rearrange("p (q two) -> p q two", two=2)
            t64 = consts.tile([128, 64], F32)
            nc.vector.tensor_add(t64, a2[:, :, 0], a2[:, :, 1])
            b2v = t64.rearrange("p (q two) -> p q two", two=2)
            t32 = consts.tile([128, 32], F32)
            nc.vector.tensor_add(t32, b2v[:, :, 0], b2v[:, :, 1])
            ones_lo = consts.tile([128, 64], BF16)
            nc.vector.memset(ones_lo, 0)
            nc.scalar.copy(out=ones_lo[:, 0:32], in_=t32)
            ones_hi = consts.tile([128, 64], BF16)
            nc.vector.memset(ones_hi, 0)
            nc.scalar.copy(out=ones_hi[:, 32:64], in_=t32)

            # ---- weights prep (one-time) ----
            w1_sb = consts.tile([DIM, D4], F32)
            nc.sync.dma_start(out=w1_sb, in_=W1_d[:, :])
            w1t_ps = pp_m.tile([D4, DIM], F32, tag="ps1")
            nc.tensor.transpose(w1t_ps, w1_sb, ident[:DIM, :DIM])
            W1T = consts.tile([D4, DIM], BF16)
            nc.scalar.copy(out=W1T, in_=w1t_ps)

            W2T_f = consts.tile([DIM, 1], F32)
            nc.sync.dma_start(out=W2T_f, in_=W2_d.rearrange("a b -> b a"))
            # block-diag W2^T [128, 4]: col j = W2^T at partitions 32j..
            w2bd_f = consts.tile([128, EV], F32)
            nc.vector.memset(w2bd_f, 0)
            for j in range(EV):
                nc.scalar.copy(
                    out=w2bd_f[j * DIM : (j + 1) * DIM, j : j + 1], in_=W2T_f
                )
            W2BD = consts.tile([128, EV], BF16)
            nc.scalar.copy(out=W2BD, in_=w2bd_f)
            b1T = consts.tile([DIM, 1], F32)
            nc.sync.dma_start(out=b1T, in_=b1_d[:].unsqueeze(1))
            b1T2 = consts.tile([2 * DIM, 1], F32)
            nc.scalar.copy(out=b1T2[0:DIM, :], in_=b1T)
            nc.scalar.copy(out=b1T2[DIM : 2 * DIM, :], in_=b1T)

            WihT, WhhT, bsum_rz, bihn_t, bhhn_t = [], [], [], [], []
            for hop in range(N_HOP):
                wih_sb = consts.tile([3 * DIM, DIM], F32, tag="wload", bufs=4)
                nc.sync.dma_start(out=wih_sb, in_=Wih_d[hop])
                wt_ps = pp_m.tile([DIM, 3 * DIM], F32, tag="ps1")
                nc.tensor.transpose(wt_ps, wih_sb, ident[: 3 * DIM, : 3 * DIM])
                wT = consts.tile([DIM, 3 * DIM], F32, tag=f"wihT{hop}")
                nc.scalar.copy(out=wT, in_=wt_ps)
                WihT.append(wT)

                whh_sb = consts.tile([3 * DIM, DIM], F32, tag="wload", bufs=4)
                nc.sync.dma_start(out=whh_sb, in_=Whh_d[hop])
                wt_ps2 = pp_m.tile([DIM, 3 * DIM], F32, tag="ps1")
                nc.tensor.transpose(wt_ps2, whh_sb, ident[: 3 * DIM, : 3 * DIM])
                wT2 = consts.tile([DIM, 3 * DIM], F32, tag=f"whhT{hop}")
                nc.scalar.copy(out=wT2, in_=wt_ps2)
                WhhT.append(wT2)

                gate_b = []
                for gd, gname in ((bih_d, "ih"), (bhh_d, "hh")):
                    for gate in range(3):
                        bt = consts.tile([DIM, 1], F32, tag=f"b{gname}{hop}{gate}")
                        nc.sync.dma_start(
                            out=bt,
                            in_=gd[hop, gate * DIM : (gate + 1) * DIM].unsqueeze(1),
                        )
                        gate_b.append(bt)
                b_r = consts.tile([DIM, 1], F32, tag=f"b_r{hop}")
                nc.vector.tensor_add(b_r, gate_b[0], gate_b[3])
                b_z = consts.tile([DIM, 1], F32, tag=f"b_z{hop}")
                nc.vector.tensor_add(b_z, gate_b[1], gate_b[4])
                bsum_rz.append((b_r, b_z))
                bihn_t.append(gate_b[2])
                bhhn_t.append(gate_b[5])

            # ---- initial M state ----
            vs_row = consts.tile([BC, DIM], F32)
            nc.sync.dma_start(out=vs_row, in_=vs_d[:, :])
            vst_ps = pp_m.tile([DIM, BC], F32, tag="ps1")
            nc.tensor.transpose(vst_ps, vs_row, ident)
            vsT = consts.tile([DIM, BC], F32)
            nc.scalar.copy(out=vsT, in_=vst_ps)
            MT = vsT  # current M^T [d, b]

            # v_rep [b, (m16, d)] bf16 via packed log-doubling
            v_rep = consts.tile([BC, MC * DIM], BF16)
            nc.vector.tensor_copy(v_rep[:, 0:DIM], vs_row)
            w = DIM
            while w < MC * DIM:
                nc.vector.tensor_copy(v_rep[:, w : 2 * w], v_rep[:, 0:w])
                w *= 2

            M_rep = v_rep  # hop 0: M == vs

            for hop in range(N_HOP):
                # h for the whole hop+sblk: [p=(q,e4), (eg, m)]
                h_sb = []
                for S in range(NSB):
                    h_t = hop_io.tile([128, NEG * N_MEM], BF16, tag="h", bufs=4)
                    # sync queue: Act's HWDGE queue is busy during tails
                    nc.sync.dma_start(out=h_t, in_=hs_d[hop, S])
                    h_sb.append(h_t)
                t_hop = hop_io.tile([BC, N_MEM * DIM], BF16, tag="t_hop")
                nc.sync.dma_start(
                    out=t_hop, in_=ts_d[:, hop].rearrange("b m d -> b (m d)")
                )

                # Rh accumulators: [b=128, (m16, d32)] f32, one per m-chunk
                rh_ps = [
                    pp_rh.tile([128, MC * DIM], F32, tag=f"rh{c}", name=f"rh{c}")
                    for c in range(NCH)
                ]

                # ---- einsum: Rh = sum_e R*h via PE block-diag reduce ----
                for S in range(NSB):
                    half = (S // 2) * 64
                    ones_bd = ones_lo if S % 2 == 0 else ones_hi
                    for quad in range(NEG // 4):
                        P_ts = []
                        A01 = A23 = None
                        for e4i in range(4):
                            eg = quad * 4 + e4i
                            r_t = rpool.tile(
                                [128, DIM * N_MEM], BF16, tag="R", bufs=12
                            )
                            nc.sync.dma_start(out=r_t, in_=Rs_d[hop, S, eg])
                            if variant == "dma":
                                continue
                            # single DVE broadcast-mul (h stride-0 over d,
                            # packed inner m): ~750 ns measured on HW
                            P_t = ppool.tile(
                                [128, DIM * N_MEM], BF16, tag="P", bufs=6
                            )
                            h_sl = h_sb[S][
                                :, eg * N_MEM : (eg + 1) * N_MEM
                            ]
                            h_b = h_sl.unsqueeze(1).broadcast_to(
                                (128, DIM, N_MEM)
                            )
                            nc.vector.tensor_tensor(
                                P_t.rearrange("p (d m) -> p d m", d=DIM),
                                r_t.rearrange("p (d m) -> p d m", d=DIM),
                                h_b,
                                op=ALU.mult,
                            )
                            P_ts.append(P_t)
                            if variant == "nomm":
                                continue
                            # quad-sum tree on DVE, emitted as operands land
                            if e4i == 1:
                                A01 = ppool.tile(
                                    [128, DIM * N_MEM], BF16, tag="PA", bufs=3
                                )
                                nc.vector.tensor_add(A01, P_ts[0], P_ts[1])
                            elif e4i == 3:
                                A23 = ppool.tile(
                                    [128, DIM * N_MEM], BF16, tag="PB", bufs=3
                                )
                                nc.vector.tensor_add(A23, P_ts[2], P_ts[3])
                        if variant in ("dma", "nomm"):
                            continue
                        AQ = ppool.tile([128, DIM * N_MEM], BF16, tag="PQ",
                                        bufs=3)
                        nc.vector.tensor_add(AQ, A01, A23)
                        Pm = AQ.rearrange("p (d m) -> p m d", d=DIM)
                        for c in range(NCH):
                            nc.tensor.matmul(
                                rh_ps[c][half : half + 64, :],
                                lhsT=ones_bd,
                                rhs=Pm[:, c * MC : (c + 1) * MC, :],
                                start=(S % 2 == 0 and quad == 0),
                                stop=(S % 2 == 1 and quad == NEG // 4 - 1),
                            )

                if variant in ("dma", "nomm"):
                    continue
                if variant == "notail":
                    # consume rh into out cheaply to keep deps
                    if hop == N_HOP - 1:
                        M_row = mstate.tile([BC, DIM], F32, tag="M_row")
                        nc.scalar.copy(out=M_row, in_=rh_ps[0][:, 0:DIM])
                        nc.sync.dma_start(out=out_d[:, :], in_=M_row)
                    else:
                        for c in range(NCH):
                            rh_sb = fpool.tile([BC, MC * DIM], BF16, tag="rh_sb")
                            nc.scalar.copy(out=rh_sb, in_=rh_ps[c])
                    continue

                # ---- features z = [Rh*v, Rh*M, |Rh-v|, |Rh-M|] ----
                z_hop = zpool.tile([BC, N_MEM * 4 * DIM], BF16, tag="z")
                z4 = z_hop.rearrange("b (m f d) -> b m f d", f=4, d=DIM)
                vr3 = v_rep.rearrange("b (m d) -> b m d", d=DIM)
                mr3 = M_rep.rearrange("b (m d) -> b m d", d=DIM)
                for c in range(NCH):
                    mc = slice(c * MC, (c + 1) * MC)
                    rh_sb = fpool.tile([BC, MC * DIM], BF16, tag="rh_sb")
                    if c % 2 == 0:
                        nc.scalar.copy(out=rh_sb, in_=rh_ps[c])
                    else:
                        nc.vector.tensor_copy(rh_sb, rh_ps[c])
                    rh3 = rh_sb.rearrange("b (m d) -> b m d", d=DIM)
                    nc.vector.tensor_mul(z4[:, mc, 0, :], rh3, vr3)
                    nc.vector.tensor_mul(z4[:, mc, 1, :], rh3, mr3)
                    nc.gpsimd.tensor_tensor(
                        z4[:, mc, 2, :], rh3, vr3, op=ALU.subtract
                    )
                    nc.gpsimd.tensor_tensor(
                        z4[:, mc, 3, :], rh3, mr3, op=ALU.subtract
                    )
                    nc.scalar.activation(
                        z4[:, mc, 2:4, :], z4[:, mc, 2:4, :], AF.Abs
                    )

                # ---- MLP per m4-group: transpose + matmuls; groups of 4
                # stack a1 into [128, 512] (32-aligned partition offsets) so
                # one block-diag W2 matmul emits Z for 4 groups at once.
                # Final Z col-order is m' = (m4, G, g4); ts is host-permuted
                # to match (softmax is order-invariant).
                zT_ps = pp_m.tile([BC, N_MEM], F32, tag="ps1")
                zf = z_hop.rearrange("b (m fd) -> b m fd", fd=4 * DIM)
                for G in range(EV):
                    a1_4 = apool.tile([128, EV * BC], BF16, tag="a1")
                    for g4 in range(EV):
                        g = G * EV + g4
                        zt_ps = pp_zt.tile([D4, EV * BC], BF16, tag="zt")
                        for j in range(EV):
                            nc.tensor.transpose(
                                zt_ps[:, j * BC : (j + 1) * BC],
                                zf[:, g * EV + j, :],
                                ident16,
                            )
                        zt_sb = zpool.tile(
                            [D4, EV * BC], BF16, tag="zt_sb", bufs=3
                        )
                        if g % 2 == 0:
                            nc.scalar.copy(out=zt_sb, in_=zt_ps)
                        else:
                            nc.vector.tensor_copy(zt_sb, zt_ps)
                        ps1 = pp_m.tile([DIM, EV * BC], F32, tag="ps1")
                        nc.tensor.matmul(
                            ps1, lhsT=W1T, rhs=zt_sb, start=True, stop=True
                        )
                        nc.scalar.activation(
                            a1_4[g4 * DIM : (g4 + 1) * DIM, :],
                            ps1,
                            AF.Tanh,
                            bias=b1T,
                        )
                    ps2 = pp_2.tile([EV, EV * BC], F32, tag="ps2")
                    nc.tensor.matmul(ps2, lhsT=W2BD, rhs=a1_4, start=True, stop=True)
                    z4sb = zpool.tile([EV, EV * BC], F32, tag="z4sb", bufs=2)
                    if G % 2 == 0:
                        nc.scalar.copy(out=z4sb, in_=ps2)
                    else:
                        nc.vector.tensor_copy(z4sb, ps2)
                    for j in range(EV):
                        nc.tensor.transpose(
                            zT_ps[:, j * MC + G * EV : j * MC + (G + 1) * EV],
                            z4sb[:, j * BC : (j + 1) * BC],
                            ident[:EV, :EV],
                        )

                # softmax over m (skip max-subtract; |Z| tanh-bounded),
                # normalize o after the t-reduction
                e_row = small.tile([BC, N_MEM], F32, tag="e_row")
                nc.scalar.activation(e_row, zT_ps, AF.Exp)
                e16 = small.tile([BC, N_MEM], BF16, tag="e16")
                nc.scalar.copy(out=e16, in_=e_row)
                ssum = small.tile([BC, 1], F32, tag="ssum")
                nc.vector.tensor_reduce(out=ssum, in_=e_row, axis=AX.X, op=ALU.add)
                rsum = small.tile([BC, 1], F32, tag="rsum")
                nc.vector.reciprocal(rsum, ssum)

                # o[b,d] = (sum_m t[b,m,d] * e[b,m]) * rsum[b]
                t3 = t_hop.rearrange("b (m d) -> b m d", d=DIM)
                g3 = e16.unsqueeze(2).broadcast_to((BC, N_MEM, DIM))
                nc.vector.tensor_tensor(t3, t3, g3, op=ALU.mult)
                for mh in (32, 16, 8, 4, 2):
                    nc.vector.tensor_add(
                        t3[:, :mh, :], t3[:, :mh, :], t3[:, mh : 2 * mh, :]
                    )
                o_raw = small.tile([BC, DIM], F32, tag="o_raw")
                nc.vector.tensor_add(
                    o_raw.unsqueeze(1), t3[:, 0:1, :], t3[:, 1:2, :]
                )
                o_row = small.tile([BC, DIM], F32, tag="o_row")
                nc.vector.tensor_scalar_mul(o_row, o_raw, rsum)

                # ---- GRU (transposed layout [*, b], f32) ----
                ot_ps = pp_m.tile([DIM, BC], F32, tag="ps1")
                nc.tensor.transpose(ot_ps, o_row, ident)
                oT = small.tile([DIM, BC], F32, tag="oT")
                nc.scalar.copy(out=oT, in_=ot_ps)

                def gate_pair(g):
                    gi = pp_m.tile([DIM, BC], F32, tag="ps1")
                    nc.tensor.matmul(
                        gi,
                        lhsT=WihT[hop][:, g * DIM : (g + 1) * DIM],
                        rhs=oT,
                        start=True,
                        stop=True,
                    )
                    gh = pp_m.tile([DIM, BC], F32, tag="ps1")
                    nc.tensor.matmul(
                        gh,
                        lhsT=WhhT[hop][:, g * DIM : (g + 1) * DIM],
                        rhs=MT,
                        start=True,
                        stop=True,
                    )
                    return gi, gh

                rz_t = []
                for g in range(2):
                    gi, gh = gate_pair(g)
                    gh_sb = small.tile([DIM, BC], F32, tag=f"gh{g}sb")
                    nc.vector.tensor_copy(gh_sb, gh)
                    gb = small.tile([DIM, BC], F32, tag=f"g{g}b")
                    # (gi + b) + gh in one DVE op (only one PSUM operand)
                    nc.vector.scalar_tensor_tensor(
                        out=gb, in0=gi, scalar=bsum_rz[hop][g], in1=gh_sb,
                        op0=ALU.add, op1=ALU.add,
                    )
                    gt = small.tile([DIM, BC], F32, tag=f"gate{g}")
                    nc.scalar.activation(gt, gb, AF.Sigmoid)
                    rz_t.append(gt)
                r_t, z_t = rz_t

                gi_n, gh_n = gate_pair(2)
                n1 = small.tile([DIM, BC], F32, tag="n1")
                # (gh_n + b_hh_n) * r
                nc.vector.scalar_tensor_tensor(
                    out=n1, in0=gh_n, scalar=bhhn_t[hop], in1=r_t,
                    op0=ALU.add, op1=ALU.mult,
                )
                n2 = small.tile([DIM, BC], F32, tag="n2")
                # (gi_n + b_ih_n) + n1
                nc.vector.scalar_tensor_tensor(
                    out=n2, in0=gi_n, scalar=bihn_t[hop], in1=n1,
                    op0=ALU.add, op1=ALU.add,
                )
                n_t = small.tile([DIM, BC], F32, tag="n_t")
                nc.scalar.activation(n_t, n2, AF.Tanh)

                # M' = n + z * (M - n)
                MT_new = mstate.tile([DIM, BC], F32, tag="MT")
                nc.vector.tensor_sub(MT_new, MT, n_t)
                nc.vector.tensor_mul(MT_new, MT_new, z_t)
                nc.vector.tensor_add(MT_new, MT_new, n_t)
                MT = MT_new

                mrow_ps = pp_m.tile([BC, DIM], F32, tag="ps1")
                nc.tensor.transpose(mrow_ps, MT, ident[:DIM, :DIM])
                if hop < N_HOP - 1:
                    # M_rep [b, (m16, d)] bf16 via packed log-doubling
                    M_rep_new = mstate.tile([BC, MC * DIM], BF16, tag="M_rep")
                    nc.scalar.copy(out=M_rep_new[:, 0:DIM], in_=mrow_ps)
                    w = DIM
                    while w < MC * DIM:
                        nc.vector.tensor_copy(
                            M_rep_new[:, w : 2 * w], M_rep_new[:, 0:w]
                        )
                        w *= 2
                    M_rep = M_rep_new
                else:
                    M_row = mstate.tile([BC, DIM], F32, tag="M_row")
                    nc.scalar.copy(out=M_row, in_=mrow_ps)
                    nc.sync.dma_start(out=out_d[:, :], in_=M_row)

    nc.compile()
    return nc


_NC_CACHE = None


def _get_nc():
    global _NC_CACHE
    if _NC_CACHE is None:
        _NC_CACHE = build_nc()
    return _NC_CACHE


def _bf16(x):
    import ml_dtypes

    return np.asarray(x).astype(ml_dtypes.bfloat16)


def _fp8(x):
    import ml_dtypes

    return np.asarray(x).astype(ml_dtypes.float8_e4m3)


def permute_R(x):
    """Rs [BC, N_HOP, m, d, e] -> [hop, S, eg, (q,e4), (d,m)] bf16."""
    y = x.reshape(NSB, QB, N_HOP, N_MEM, DIM, NEG, EV)
    # [S, q, hop, m, d, eg, e4] -> [hop, S, eg, q, e4, d, m]
    y = y.transpose(2, 0, 5, 1, 6, 4, 3).reshape(
        N_HOP, NSB, NEG, 128, DIM * N_MEM
    )
    return np.ascontiguousarray(_bf16(y))


def permute_h(x):
    """hs [BC, N_HOP, m, e] -> [hop, S, (q,e4), (eg,m)] bf16."""
    y = x.reshape(NSB, QB, N_HOP, N_MEM, NEG, EV)
    # [S, q, hop, m, eg, e4] -> [hop, S, q, e4, eg, m]
    y = y.transpose(2, 0, 1, 5, 4, 3)
    return np.ascontiguousarray(y.reshape(N_HOP, NSB, 128, NEG * N_MEM))


def permute_t(x):
    """ts [BC, hop, m, d]: m reordered to m' = (m4, G, g4), m = (4G+g4)*4+m4."""
    y = x.reshape(BC, N_HOP, EV, EV, EV, DIM)  # [b, hop, G, g4, m4, d]
    return np.ascontiguousarray(
        y.transpose(0, 1, 4, 2, 3, 5).reshape(BC, N_HOP, N_MEM, DIM)
    )


def make_in_maps(hs, Rs, ts, vs, W1, b1, W2, W_ih, W_hh, b_ih, b_hh):
    in_maps = []
    for c in range(N_CORES):
        sl = slice(c * BC, (c + 1) * BC)
        in_maps.append(
            {
                "Rs": permute_R(Rs[sl]),
                "hs": permute_h(_bf16(hs[sl])),
                "ts": permute_t(_bf16(ts[sl])),
                "vs": np.ascontiguousarray(vs[sl]),
                "W1": np.ascontiguousarray(W1),
                "b1": np.ascontiguousarray(b1),
                "W2": np.ascontiguousarray(W2),
                "W_ih": np.ascontiguousarray(W_ih),
                "W_hh": np.ascontiguousarray(W_hh),
                "b_ih": np.ascontiguousarray(b_ih),
                "b_hh": np.ascontiguousarray(b_hh),
            }
        )
    return in_maps


def kernel(hs, Rs, ts, vs, W1, b1, W2, b2, W_ih, W_hh, b_ih, b_hh):
    from concourse.bass_utils import run_bass_kernel_spmd

    nc = _get_nc()
    in_maps = make_in_maps(hs, Rs, ts, vs, W1, b1, W2, W_ih, W_hh, b_ih, b_hh)
    res = run_bass_kernel_spmd(nc, in_maps, list(range(N_CORES)))
    return np.concatenate([r["out"] for r in res.results], axis=0)


# revision 72
# speedup vs baseline: 1.1916x; 1.0127x over previous
"""EpisodicMemory kernel for Trainium2, 8-core data-parallel. v2.

Reference computation (per batch b, d=32, m=64 memory slots, 2 hops):
    M = vs[b]
    for hop:
        Rh[m,:] = R[b,hop,m] @ h[b,hop,m]                  # batched matvec
        z = [Rh*v, Rh*M, |Rh-v|, |Rh-M|]                   # [m, 4d]
        Z = tanh(z @ W1.T + b1) @ W2.T (+ b2: dropped — softmax-invariant)
        g = softmax(Z over m); o = sum_m ts[b,hop,m] * g[m]
        M = GRUCell(o, M)
    out[b] = M

Sharding: pure data parallel over batch; 128 batches per core.

v2 design vs v1 (396 us -> 236 us), all choices HW-measured:
  - Einsum partition layout p=(q32, e4): 32 batches x 4 e-values.  R is
    host-permuted to [hop, S, eg, (q,e4), (d,m)] bf16 (dense 512 KB DMAs
    reach ~385 GB/s; fp8 was tried and rejected -- 1-byte operands drop
    DVE to the slow path, costing more than the DMA it saves).
  - P = R*h is ONE DVE broadcast-mul per e-group tile (h stride-0 over d,
    packed inner m: ~750 ns measured -- faster than packed h_rep
    materialization, contrary to the v1 session's note; Pool is 5x
    slower and gets none of the muls).
  - e-reduction on the TensorEngine: a quad of e-group P-tiles is first
    summed on DVE (3 adds, ~800 ns each; each add kills 4 PE matmuls),
    then contracted with a static block-diag-ones lhsT [128, 64]
    accumulating over 2 quads into [128, 512] PSUM chunks.  Super-blocks
    write 64-row halves (PSUM matmul outputs can only start at partition
    0/32/64).  Rh lands as [b=128, (m16, d32)] chunks, feature-ready.
  - Features: f0/f1 muls on DVE, f2/f3 subs on Pool (strided 512-f ops
    measured ~808 ns on Pool vs 1288 on DVE), abs on Act.
  - MLP: PE transpose z per m to z^T [feat*d, (m4, b)], W1 matmul + tanh
    into a [128, 512] a1 stack (32-aligned partition offsets), then one
    block-diag W2 matmul emits Z for 4 m-groups at once; Z returns to
    [b, m] via tiny PE transposes -- no DRAM bounce anywhere (the v1/v2.0
    Z gather DMA was descriptor-pathological: 4-byte partition stride).
  - Softmax skips the max-subtract (|Z| tanh-bounded), o is normalized
    after the t-reduction; GRU in transposed [d, b] layout with
    scalar_tensor_tensor-fused bias adds.
  - v_rep / M_rep are [b, (m16, d)] tiles built with a few packed SBUF
    copies -- the v1 DRAM broadcast bounce is gone.

Measured dead ends (do not retry without new evidence): fp8 R loads
(DVE 1-byte slow path), Pool-heavy muls/features, packed h_rep, stacked
2-group tanh, max-form abs, all-DVE PSUM copies, emitting both hops'
einsums before the tails (delays the serial GRU chain), pre-add depths
1 and 3 (depth 2 is the optimum), R DMAs split onto the Act HWDGE queue
(transfers contend with Act's tail compute), deeper R/P buffering,
h/t DMAs on the sync queue (242.5 us -- within noise, not better).
Untried with upside: a cheap fp8 P-producer to unlock PE DoubleRow
reduction; sub-hop software pipelining of einsum vs tail.
"""

import numpy as np

import concourse.bacc as bacc
import concourse.bass as bass
import concourse.mybir as mybir
import concourse.tile as tile
from concourse.masks import make_identity

F32 = mybir.dt.float32
BF16 = mybir.dt.bfloat16
FP8 = mybir.dt.float8e4
AF = mybir.ActivationFunctionType
ALU = mybir.AluOpType
AX = mybir.AxisListType

B, N_HOP, N_MEM, DIM = 1024, 2, 64, 32
N_CORES = 8
BC = B // N_CORES            # 128 batches per core
NSB = 4                      # super-blocks per core
QB = 32                      # batches per super-block (partition q-dim)
EV = 4                       # e-values per partition group
NEG = DIM // EV              # 8 e-groups
MC = 16                      # m per PE output chunk
NCH = N_MEM // MC            # 4 chunks
D4 = 4 * DIM                 # 128 MLP input features




def build_nc(n_iter: int = 1, variant: str = "full") -> bass.Bass:
    """variant: 'full' | 'dma' (loads only)"""
    nc = bacc.Bacc("TRN2")

    # host-permuted layouts (b = S*32 + q within a core):
    #   Rs[hop, S, eg, (q,e4), (d,m)]   bf16
    #   hs[hop, S, (q,e4), (eg,m)]      bf16
    Rs_d = nc.dram_tensor(
        "Rs", [N_HOP, NSB, NEG, 128, DIM * N_MEM], BF16, kind="ExternalInput"
    )
    hs_d = nc.dram_tensor(
        "hs", [N_HOP, NSB, 128, NEG * N_MEM], BF16, kind="ExternalInput"
    )
    ts_d = nc.dram_tensor("ts", [BC, N_HOP, N_MEM, DIM], BF16, kind="ExternalInput")
    vs_d = nc.dram_tensor("vs", [BC, DIM], F32, kind="ExternalInput")
    W1_d = nc.dram_tensor("W1", [DIM, D4], F32, kind="ExternalInput")
    b1_d = nc.dram_tensor("b1", [DIM], F32, kind="ExternalInput")
    W2_d = nc.dram_tensor("W2", [1, DIM], F32, kind="ExternalInput")
    Wih_d = nc.dram_tensor("W_ih", [N_HOP, 3 * DIM, DIM], F32, kind="ExternalInput")
    Whh_d = nc.dram_tensor("W_hh", [N_HOP, 3 * DIM, DIM], F32, kind="ExternalInput")
    bih_d = nc.dram_tensor("b_ih", [N_HOP, 3 * DIM], F32, kind="ExternalInput")
    bhh_d = nc.dram_tensor("b_hh", [N_HOP, 3 * DIM], F32, kind="ExternalInput")
    out_d = nc.dram_tensor("out", [BC, DIM], F32, kind="ExternalOutput")

    import contextlib

    with tile.TileContext(nc) as tc:
        with (
            (tc.For_i(0, n_iter, 1) if n_iter > 1 else contextlib.nullcontext()),
            tc.tile_pool(name="consts", bufs=1) as consts,
            tc.tile_pool(name="hop_io", bufs=2) as hop_io,
            tc.tile_pool(name="rpool", bufs=12) as rpool,
            tc.tile_pool(name="ppool", bufs=2) as ppool,
            tc.tile_pool(name="fpool", bufs=2) as fpool,
            tc.tile_pool(name="zpool", bufs=2) as zpool,
            tc.tile_pool(name="apool", bufs=2) as apool,
            tc.tile_pool(name="small", bufs=2) as small,
            tc.tile_pool(name="mstate", bufs=2) as mstate,
            tc.tile_pool(name="pp_rh", bufs=1, space="PSUM") as pp_rh,
            tc.tile_pool(name="pp_zt", bufs=1, space="PSUM") as pp_zt,
            tc.tile_pool(name="pp_m", bufs=2, space="PSUM") as pp_m,
            tc.tile_pool(name="pp_2", bufs=1, space="PSUM") as pp_2,
        ):
            ident = consts.tile([128, 128], F32)
            make_identity(nc, ident)
            ident16 = consts.tile([128, 128], BF16)
            nc.scalar.copy(out=ident16, in_=ident)

            # block-diag ones [p=(q,e4), q]: 1 iff p//4 == q.  PSUM matmul
            # outputs may only start at partition 0/32/64, so super-blocks
            # write 64-row halves: lo has the diag in cols 0-31 (S even),
            # hi in cols 32-63 (S odd); the other half-block's rows get +0.
            a2 = ident.rearrange("p (q two) -> p q two", two=2)
            t64 = consts.tile([128, 64], F32)
            nc.vector.tensor_add(t64, a2[:, :, 0], a2[:, :, 1])
            b2v = t64.rearrange("p (q two) -> p q two", two=2)
            t32 = consts.tile([128, 32], F32)
            nc.vector.tensor_add(t32, b2v[:, :, 0], b2v[:, :, 1])
            ones_lo = consts.tile([128, 64], BF16)
            nc.vector.memset(ones_lo, 0)
            nc.scalar.copy(out=ones_lo[:, 0:32], in_=t32)
            ones_hi = consts.tile([128, 64], BF16)
            nc.vector.memset(ones_hi, 0)
            nc.scalar.copy(out=ones_hi[:, 32:64], in_=t32)

            # ---- weights prep (one-time) ----
            w1_sb = consts.tile([DIM, D4], F32)
            nc.sync.dma_start(out=w1_sb, in_=W1_d[:, :])
            w1t_ps = pp_m.tile([D4, DIM], F32, tag="ps1")
            nc.tensor.transpose(w1t_ps, w1_sb, ident[:DIM, :DIM])
            W1T = consts.tile([D4, DIM], BF16)
            nc.scalar.copy(out=W1T, in_=w1t_ps)

            W2T_f = consts.tile([DIM, 1], F32)
            nc.sync.dma_start(out=W2T_f, in_=W2_d.rearrange("a b -> b a"))
            # block-diag W2^T [128, 4]: col j = W2^T at partitions 32j..
            w2bd_f = consts.tile([128, EV], F32)
            nc.vector.memset(w2bd_f, 0)
            for j in range(EV):
                nc.scalar.copy(
                    out=w2bd_f[j * DIM : (j + 1) * DIM, j : j + 1], in_=W2T_f
                )
            W2BD = consts.tile([128, EV], BF16)
            nc.scalar.copy(out=W2BD, in_=w2bd_f)
            b1T = consts.tile([DIM, 1], F32)
            nc.sync.dma_start(out=b1T, in_=b1_d[:].unsqueeze(1))
            b1T2 = consts.tile([2 * DIM, 1], F32)
            nc.scalar.copy(out=b1T2[0:DIM, :], in_=b1T)
            nc.scalar.copy(out=b1T2[DIM : 2 * DIM, :], in_=b1T)

            WihT, WhhT, bsum_rz, bihn_t, bhhn_t = [], [], [], [], []
            for hop in range(N_HOP):
                wih_sb = consts.tile([3 * DIM, DIM], F32, tag="wload", bufs=4)
                nc.sync.dma_start(out=wih_sb, in_=Wih_d[hop])
                wt_ps = pp_m.tile([DIM, 3 * DIM], F32, tag="ps1")
                nc.tensor.transpose(wt_ps, wih_sb, ident[: 3 * DIM, : 3 * DIM])
                wT = consts.tile([DIM, 3 * DIM], F32, tag=f"wihT{hop}")
                nc.scalar.copy(out=wT, in_=wt_ps)
                WihT.append(wT)

                whh_sb = consts.tile([3 * DIM, DIM], F32, tag="wload", bufs=4)
                nc.sync.dma_start(out=whh_sb, in_=Whh_d[hop])
                wt_ps2 = pp_m.tile([DIM, 3 * DIM], F32, tag="ps1")
                nc.tensor.transpose(wt_ps2, whh_sb, ident[: 3 * DIM, : 3 * DIM])
                wT2 = consts.tile([DIM, 3 * DIM], F32, tag=f"whhT{hop}")
                nc.scalar.copy(out=wT2, in_=wt_ps2)
                WhhT.append(wT2)

                gate_b = []
                for gd, gname in ((bih_d, "ih"), (bhh_d, "hh")):
                    for gate in range(3):
                        bt = consts.tile([DIM, 1], F32, tag=f"b{gname}{hop}{gate}")
                        nc.sync.dma_start(
                            out=bt,
                            in_=gd[hop, gate * DIM : (gate + 1) * DIM].unsqueeze(1),
                        )
                        gate_b.append(bt)
                b_r = consts.tile([DIM, 1], F32, tag=f"b_r{hop}")
                nc.vector.tensor_add(b_r, gate_b[0], gate_b[3])
                b_z = consts.tile([DIM, 1], F32, tag=f"b_z{hop}")
                nc.vector.tensor_add(b_z, gate_b[1], gate_b[4])
                bsum_rz.append((b_r, b_z))
                bihn_t.append(gate_b[2])
                bhhn_t.append(gate_b[5])

            # ---- initial M state ----
            vs_row = consts.tile([BC, DIM], F32)
            nc.sync.dma_start(out=vs_row, in_=vs_d[:, :])
            vst_ps = pp_m.tile([DIM, BC], F32, tag="ps1")
            nc.tensor.transpose(vst_ps, vs_row, ident)
            vsT = consts.tile([DIM, BC], F32)
            nc.scalar.copy(out=vsT, in_=vst_ps)
            MT = vsT  # current M^T [d, b]

            # v_rep [b, (m16, d)] bf16 via packed log-doubling
            v_rep = consts.tile([BC, MC * DIM], BF16)
            nc.vector.tensor_copy(v_rep[:, 0:DIM], vs_row)
            w = DIM
            while w < MC * DIM:
                nc.vector.tensor_copy(v_rep[:, w : 2 * w], v_rep[:, 0:w])
                w *= 2

            M_rep = v_rep  # hop 0: M == vs

            for hop in range(N_HOP):
                # h for the whole hop+sblk: [p=(q,e4), (eg, m)]
                h_sb = []
                for S in range(NSB):
                    h_t = hop_io.tile([128, NEG * N_MEM], BF16, tag="h", bufs=4)
                    nc.scalar.dma_start(out=h_t, in_=hs_d[hop, S])
                    h_sb.append(h_t)
                t_hop = hop_io.tile([BC, N_MEM * DIM], BF16, tag="t_hop")
                nc.scalar.dma_start(
                    out=t_hop, in_=ts_d[:, hop].rearrange("b m d -> b (m d)")
                )

                # Rh accumulators: [b=128, (m16, d32)] f32, one per m-chunk
                rh_ps = [
                    pp_rh.tile([128, MC * DIM], F32, tag=f"rh{c}", name=f"rh{c}")
                    for c in range(NCH)
                ]

                # ---- einsum: Rh = sum_e R*h via PE block-diag reduce ----
                for S in range(NSB):
                    half = (S // 2) * 64
                    ones_bd = ones_lo if S % 2 == 0 else ones_hi
                    for quad in range(NEG // 4):
                        P_ts = []
                        A01 = A23 = None
                        for e4i in range(4):
                            eg = quad * 4 + e4i
                            r_t = rpool.tile(
                                [128, DIM * N_MEM], BF16, tag="R", bufs=12
                            )
                            nc.sync.dma_start(out=r_t, in_=Rs_d[hop, S, eg])
                            if variant == "dma":
                                continue
                            # single DVE broadcast-mul (h stride-0 over d,
                            # packed inner m): ~750 ns measured on HW
                            P_t = ppool.tile(
                                [128, DIM * N_MEM], BF16, tag="P", bufs=6
                            )
                            h_sl = h_sb[S][
                                :, eg * N_MEM : (eg + 1) * N_MEM
                            ]
                            h_b = h_sl.unsqueeze(1).broadcast_to(
                                (128, DIM, N_MEM)
                            )
                            nc.vector.tensor_tensor(
                                P_t.rearrange("p (d m) -> p d m", d=DIM),
                                r_t.rearrange("p (d m) -> p d m", d=DIM),
                                h_b,
                                op=ALU.mult,
                            )
                            P_ts.append(P_t)
                            if variant == "nomm":
                                continue
                            # quad-sum tree on DVE, emitted as operands land
                            if e4i == 1:
                                A01 = ppool.tile(
                                    [128, DIM * N_MEM], BF16, tag="PA", bufs=3
                                )
                                nc.vector.tensor_add(A01, P_ts[0], P_ts[1])
                            elif e4i == 3:
                                A23 = ppool.tile(
                                    [128, DIM * N_MEM], BF16, tag="PB", bufs=3
                                )
                                nc.vector.tensor_add(A23, P_ts[2], P_ts[3])
                        if variant in ("dma", "nomm"):
                            continue
                        AQ = ppool.tile([128, DIM * N_MEM], BF16, tag="PQ",
                                        bufs=3)
                        nc.vector.tensor_add(AQ, A01, A23)
                        Pm = AQ.rearrange("p (d m) -> p m d", d=DIM)
                        for c in range(NCH):
                            nc.tensor.matmul(
                                rh_ps[c][half : half + 64, :],
                                lhsT=ones_bd,
                                rhs=Pm[:, c * MC : (c + 1) * MC, :],
                                start=(S % 2 == 0 and quad == 0),
                                stop=(S % 2 == 1 and quad == NEG // 4 - 1),
                            )

                if variant in ("dma", "nomm"):
                    continue
                if variant == "notail":
                    # consume rh into out cheaply to keep deps
                    if hop == N_HOP - 1:
                        M_row = mstate.tile([BC, DIM], F32, tag="M_row")
                        nc.scalar.copy(out=M_row, in_=rh_ps[0][:, 0:DIM])
                        nc.sync.dma_start(out=out_d[:, :], in_=M_row)
                    else:
                        for c in range(NCH):
                            rh_sb = fpool.tile([BC, MC * DIM], BF16, tag="rh_sb")
                            nc.scalar.copy(out=rh_sb, in_=rh_ps[c])
                    continue

                # ---- features z = [Rh*v, Rh*M, |Rh-v|, |Rh-M|] ----
                z_hop = zpool.tile([BC, N_MEM * 4 * DIM], BF16, tag="z")
                z4 = z_hop.rearrange("b (m f d) -> b m f d", f=4, d=DIM)
                vr3 = v_rep.rearrange("b (m d) -> b m d", d=DIM)
                mr3 = M_rep.rearrange("b (m d) -> b m d", d=DIM)
                for c in range(NCH):
                    mc = slice(c * MC, (c + 1) * MC)
                    rh_sb = fpool.tile([BC, MC * DIM], BF16, tag="rh_sb")
                    if c % 2 == 0:
                        nc.scalar.copy(out=rh_sb, in_=rh_ps[c])
                    else:
                        nc.vector.tensor_copy(rh_sb, rh_ps[c])
                    rh3 = rh_sb.rearrange("b (m d) -> b m d", d=DIM)
                    nc.vector.tensor_mul(z4[:, mc, 0, :], rh3, vr3)
                    nc.vector.tensor_mul(z4[:, mc, 1, :], rh3, mr3)
                    nc.gpsimd.tensor_tensor(
                        z4[:, mc, 2, :], rh3, vr3, op=ALU.subtract
                    )
                    nc.gpsimd.tensor_tensor(
                        z4[:, mc, 3, :], rh3, mr3, op=ALU.subtract
                    )
                    nc.scalar.activation(
                        z4[:, mc, 2:4, :], z4[:, mc, 2:4, :], AF.Abs
                    )

                # ---- MLP per m4-group: transpose + matmuls; groups of 4
                # stack a1 into [128, 512] (32-aligned partition offsets) so
                # one block-diag W2 matmul emits Z for 4 groups at once.
                # Final Z col-order is m' = (m4, G, g4); ts is host-permuted
                # to match (softmax is order-invariant).
                zT_ps = pp_m.tile([BC, N_MEM], F32, tag="ps1")
                zf = z_hop.rearrange("b (m fd) -> b m fd", fd=4 * DIM)
                for G in range(EV):
                    a1_4 = apool.tile([128, EV * BC], BF16, tag="a1")
                    for g4 in range(EV):
                        g = G * EV + g4
                        zt_ps = pp_zt.tile([D4, EV * BC], BF16, tag="zt")
                        for j in range(EV):
                            nc.tensor.transpose(
                                zt_ps[:, j * BC : (j + 1) * BC],
                                zf[:, g * EV + j, :],
                                ident16,
                            )
                        zt_sb = zpool.tile(
                            [D4, EV * BC], BF16, tag="zt_sb", bufs=3
                        )
                        if g % 2 == 0:
                            nc.scalar.copy(out=zt_sb, in_=zt_ps)
                        else:
                            nc.vector.tensor_copy(zt_sb, zt_ps)
                        ps1 = pp_m.tile([DIM, EV * BC], F32, tag="ps1")
                        nc.tensor.matmul(
                            ps1, lhsT=W1T, rhs=zt_sb, start=True, stop=True
                        )
                        nc.scalar.activation(
                            a1_4[g4 * DIM : (g4 + 1) * DIM, :],
                            ps1,
                            AF.Tanh,
                            bias=b1T,
                        )
                    ps2 = pp_2.tile([EV, EV * BC], F32, tag="ps2")
                    nc.tensor.matmul(ps2, lhsT=W2BD, rhs=a1_4, start=True, stop=True)
                    z4sb = zpool.tile([EV, EV * BC], F32, tag="z4sb", bufs=2)
                    if G % 2 == 0:
                        nc.scalar.copy(out=z4sb, in_=ps2)
                    else:
                        nc.vector.tensor_copy(z4sb, ps2)
                    for j in range(EV):
                        nc.tensor.transpose(
                            zT_ps[:, j * MC + G * EV : j * MC + (G + 1) * EV],
                            z4sb[:, j * BC : (j + 1) * BC],
                            ident[:EV, :EV],
                        )

                # softmax over m (skip max-subtract; |Z| tanh-bounded),
                # normalize o after the t-reduction
                e_row = small.tile([BC, N_MEM], F32, tag="e_row")
                nc.scalar.activation(e_row, zT_ps, AF.Exp)
                e16 = small.tile([BC, N_MEM], BF16, tag="e16")
                nc.scalar.copy(out=e16, in_=e_row)
                ssum = small.tile([BC, 1], F32, tag="ssum")
                nc.vector.tensor_reduce(out=ssum, in_=e_row, axis=AX.X, op=ALU.add)
                rsum = small.tile([BC, 1], F32, tag="rsum")
                nc.vector.reciprocal(rsum, ssum)

                # o[b,d] = (sum_m t[b,m,d] * e[b,m]) * rsum[b]
                t3 = t_hop.rearrange("b (m d) -> b m d", d=DIM)
                g3 = e16.unsqueeze(2).broadcast_to((BC, N_MEM, DIM))
                nc.vector.tensor_tensor(t3, t3, g3, op=ALU.mult)
                for mh in (32, 16, 8, 4, 2):
                    nc.vector.tensor_add(
                        t3[:, :mh, :], t3[:, :mh, :], t3[:, mh : 2 * mh, :]
                    )
                o_raw = small.tile([BC, DIM], F32, tag="o_raw")
                nc.vector.tensor_add(
                    o_raw.unsqueeze(1), t3[:, 0:1, :], t3[:, 1:2, :]
                )
                o_row = small.tile([BC, DIM], F32, tag="o_row")
                nc.vector.tensor_scalar_mul(o_row, o_raw, rsum)

                # ---- GRU (transposed layout [*, b], f32) ----
                ot_ps = pp_m.tile([DIM, BC], F32, tag="ps1")
                nc.tensor.transpose(ot_ps, o_row, ident)
                oT = small.tile([DIM, BC], F32, tag="oT")
                nc.scalar.copy(out=oT, in_=ot_ps)

                def gate_pair(g):
                    gi = pp_m.tile([DIM, BC], F32, tag="ps1")
                    nc.tensor.matmul(
                        gi,
                        lhsT=WihT[hop][:, g * DIM : (g + 1) * DIM],
                        rhs=oT,
                        start=True,
                        stop=True,
                    )
                    gh = pp_m.tile([DIM, BC], F32, tag="ps1")
                    nc.tensor.matmul(
                        gh,
                        lhsT=WhhT[hop][:, g * DIM : (g + 1) * DIM],
                        rhs=MT,
                        start=True,
                        stop=True,
                    )
                    return gi, gh

                rz_t = []
                for g in range(2):
                    gi, gh = gate_pair(g)
                    gh_sb = small.tile([DIM, BC], F32, tag=f"gh{g}sb")
                    nc.vector.tensor_copy(gh_sb, gh)
                    gb = small.tile([DIM, BC], F32, tag=f"g{g}b")
                    # (gi + b) + gh in one DVE op (only one PSUM operand)
                    nc.vector.scalar_tensor_tensor(
                        out=gb, in0=gi, scalar=bsum_rz[hop][g], in1=gh_sb,
                        op0=ALU.add, op1=ALU.add,
                    )
                    gt = small.tile([DIM, BC], F32, tag=f"gate{g}")
                    nc.scalar.activation(gt, gb, AF.Sigmoid)
                    rz_t.append(gt)
                r_t, z_t = rz_t

                gi_n, gh_n = gate_pair(2)
                n1 = small.tile([DIM, BC], F32, tag="n1")
                # (gh_n + b_hh_n) * r
                nc.vector.scalar_tensor_tensor(
                    out=n1, in0=gh_n, scalar=bhhn_t[hop], in1=r_t,
                    op0=ALU.add, op1=ALU.mult,
                )
                n2 = small.tile([DIM, BC], F32, tag="n2")
                # (gi_n + b_ih_n) + n1
                nc.vector.scalar_tensor_tensor(
                    out=n2, in0=gi_n, scalar=bihn_t[hop], in1=n1,
                    op0=ALU.add, op1=ALU.add,
                )
                n_t = small.tile([DIM, BC], F32, tag="n_t")
                nc.scalar.activation(n_t, n2, AF.Tanh)

                # M' = n + z * (M - n)
                MT_new = mstate.tile([DIM, BC], F32, tag="MT")
                nc.vector.tensor_sub(MT_new, MT, n_t)
                nc.vector.tensor_mul(MT_new, MT_new, z_t)
                nc.vector.tensor_add(MT_new, MT_new, n_t)
                MT = MT_new

                mrow_ps = pp_m.tile([BC, DIM], F32, tag="ps1")
                nc.tensor.transpose(mrow_ps, MT, ident[:DIM, :DIM])
                if hop < N_HOP - 1:
                    # M_rep [b, (m16, d)] bf16 via packed log-doubling
                    M_rep_new = mstate.tile([BC, MC * DIM], BF16, tag="M_rep")
                    nc.scalar.copy(out=M_rep_new[:, 0:DIM], in_=mrow_ps)
                    w = DIM
                    while w < MC * DIM:
                        nc.vector.tensor_copy(
                            M_rep_new[:, w : 2 * w], M_rep_new[:, 0:w]
                        )
                        w *= 2
                    M_rep = M_rep_new
                else:
                    M_row = mstate.tile([BC, DIM], F32, tag="M_row")
                    nc.scalar.copy(out=M_row, in_=mrow_ps)
                    nc.sync.dma_start(out=out_d[:, :], in_=M_row)

    nc.compile()
    return nc


_NC_CACHE = None


def _get_nc():
    global _NC_CACHE
    if _NC_CACHE is None:
        _NC_CACHE = build_nc()
    return _NC_CACHE


def _bf16(x):
    import ml_dtypes

    return np.asarray(x).astype(ml_dtypes.bfloat16)


def _fp8(x):
    import ml_dtypes

    return np.asarray(x).astype(ml_dtypes.float8_e4m3)


def permute_R(x):
    """Rs [BC, N_HOP, m, d, e] -> [hop, S, eg, (q,e4), (d,m)] bf16."""
    y = x.reshape(NSB, QB, N_HOP, N_MEM, DIM, NEG, EV)
    # [S, q, hop, m, d, eg, e4] -> [hop, S, eg, q, e4, d, m]
    y = y.transpose(2, 0, 5, 1, 6, 4, 3).reshape(
        N_HOP, NSB, NEG, 128, DIM * N_MEM
    )
    return np.ascontiguousarray(_bf16(y))


def permute_h(x):
    """hs [BC, N_HOP, m, e] -> [hop, S, (q,e4), (eg,m)] bf16."""
    y = x.reshape(NSB, QB, N_HOP, N_MEM, NEG, EV)
    # [S, q, hop, m, eg, e4] -> [hop, S, q, e4, eg, m]
    y = y.transpose(2, 0, 1, 5, 4, 3)
    return np.ascontiguousarray(y.reshape(N_HOP, NSB, 128, NEG * N_MEM))


def permute_t(x):
    """ts [BC, hop, m, d]: m reordered to m' = (m4, G, g4), m = (4G+g4)*4+m4."""
    y = x.reshape(BC, N_HOP, EV, EV, EV, DIM)  # [b, hop, G, g4, m4, d]
    return np.ascontiguousarray(
        y.transpose(0, 1, 4, 2, 3, 5).reshape(BC, N_HOP, N_MEM, DIM)
    )


def make_in_maps(hs, Rs, ts, vs, W1, b1, W2, W_ih, W_hh, b_ih, b_hh):
    in_maps = []
    for c in range(N_CORES):
        sl = slice(c * BC, (c + 1) * BC)
        in_maps.append(
            {
                "Rs": permute_R(Rs[sl]),
                "hs": permute_h(_bf16(hs[sl])),
                "ts": permute_t(_bf16(ts[sl])),
                "vs": np.ascontiguousarray(vs[sl]),
                "W1": np.ascontiguousarray(W1),
                "b1": np.ascontiguousarray(b1),
                "W2": np.ascontiguousarray(W2),
                "W_ih": np.ascontiguousarray(W_ih),
                "W_hh": np.ascontiguousarray(W_hh),
                "b_ih": np.ascontiguousarray(b_ih),
                "b_hh": np.ascontiguousarray(b_hh),
            }
        )
    return in_maps


def kernel(hs, Rs, ts, vs, W1, b1, W2, b2, W_ih, W_hh, b_ih, b_hh):
    from concourse.bass_utils import run_bass_kernel_spmd

    nc = _get_nc()
    in_maps = make_in_maps(hs, Rs, ts, vs, W1, b1, W2, W_ih, W_hh, b_ih, b_hh)
    res = run_bass_kernel_spmd(nc, in_maps, list(range(N_CORES)))
    return np.concatenate([r["out"] for r in res.results], axis=0)


# revision 73
# speedup vs baseline: 1.2468x; 1.0463x over previous
"""EpisodicMemory kernel for Trainium2, 8-core data-parallel. v2.

Reference computation (per batch b, d=32, m=64 memory slots, 2 hops):
    M = vs[b]
    for hop:
        Rh[m,:] = R[b,hop,m] @ h[b,hop,m]                  # batched matvec
        z = [Rh*v, Rh*M, |Rh-v|, |Rh-M|]                   # [m, 4d]
        Z = tanh(z @ W1.T + b1) @ W2.T (+ b2: dropped — softmax-invariant)
        g = softmax(Z over m); o = sum_m ts[b,hop,m] * g[m]
        M = GRUCell(o, M)
    out[b] = M

Sharding: pure data parallel over batch; 128 batches per core.

v2 design vs v1 (396 us -> 236 us), all choices HW-measured:
  - Einsum partition layout p=(q32, e4): 32 batches x 4 e-values.  R is
    host-permuted to [hop, S, eg, (q,e4), (d,m)] bf16 (dense 512 KB DMAs
    reach ~385 GB/s; fp8 was tried and rejected -- 1-byte operands drop
    DVE to the slow path, costing more than the DMA it saves).
  - P = R*h is ONE DVE broadcast-mul per e-group tile (h stride-0 over d,
    packed inner m: ~750 ns measured -- faster than packed h_rep
    materialization, contrary to the v1 session's note; Pool is 5x
    slower and gets none of the muls).
  - e-reduction on the TensorEngine: a quad of e-group P-tiles is first
    summed on DVE (3 adds, ~800 ns each; each add kills 4 PE matmuls),
    then contracted with a static block-diag-ones lhsT [128, 64]
    accumulating over 2 quads into [128, 512] PSUM chunks.  Super-blocks
    write 64-row halves (PSUM matmul outputs can only start at partition
    0/32/64).  Rh lands as [b=128, (m16, d32)] chunks, feature-ready.
  - Features: f0/f1 muls on DVE, f2/f3 subs on Pool (strided 512-f ops
    measured ~808 ns on Pool vs 1288 on DVE), abs on Act.
  - MLP: PE transpose z per m to z^T [feat*d, (m4, b)], W1 matmul + tanh
    into a [128, 512] a1 stack (32-aligned partition offsets), then one
    block-diag W2 matmul emits Z for 4 m-groups at once; Z returns to
    [b, m] via tiny PE transposes -- no DRAM bounce anywhere (the v1/v2.0
    Z gather DMA was descriptor-pathological: 4-byte partition stride).
  - Softmax skips the max-subtract (|Z| tanh-bounded), o is normalized
    after the t-reduction; GRU in transposed [d, b] layout with
    scalar_tensor_tensor-fused bias adds.
  - v_rep / M_rep are [b, (m16, d)] tiles built with a few packed SBUF
    copies -- the v1 DRAM broadcast bounce is gone.

Measured dead ends (do not retry without new evidence): fp8 R loads
(DVE 1-byte slow path), Pool-heavy muls/features, packed h_rep, stacked
2-group tanh, max-form abs, all-DVE PSUM copies, emitting both hops'
einsums before the tails (delays the serial GRU chain), pre-add depths
1 and 3 (depth 2 is the optimum), R DMAs split onto the Act HWDGE queue
(transfers contend with Act's tail compute), deeper R/P buffering,
h/t DMAs on the sync queue (242.5 us -- within noise, not better).
Untried with upside: a cheap fp8 P-producer to unlock PE DoubleRow
reduction; sub-hop software pipelining of einsum vs tail.
"""

import numpy as np

import concourse.bacc as bacc
import concourse.bass as bass
import concourse.mybir as mybir
import concourse.tile as tile
from concourse.masks import make_identity

F32 = mybir.dt.float32
BF16 = mybir.dt.bfloat16
FP8 = mybir.dt.float8e4
AF = mybir.ActivationFunctionType
ALU = mybir.AluOpType
AX = mybir.AxisListType

B, N_HOP, N_MEM, DIM = 1024, 2, 64, 32
N_CORES = 8
BC = B // N_CORES            # 128 batches per core
NSB = 4                      # super-blocks per core
QB = 32                      # batches per super-block (partition q-dim)
EV = 4                       # e-values per partition group
NEG = DIM // EV              # 8 e-groups
MC = 16                      # m per PE output chunk
NCH = N_MEM // MC            # 4 chunks
D4 = 4 * DIM                 # 128 MLP input features




def build_nc(n_iter: int = 1, variant: str = "full") -> bass.Bass:
    """variant: 'full' | 'dma' (loads only)"""
    nc = bacc.Bacc("TRN2")

    # host-permuted layouts (b = S*32 + q within a core):
    #   Rs[hop, S, eg, (q,e4), (d,m)]   bf16
    #   hs[hop, S, (q,e4), (eg,m)]      bf16
    Rs_d = nc.dram_tensor(
        "Rs", [N_HOP, NSB, NEG, 128, DIM * N_MEM], BF16, kind="ExternalInput"
    )
    hs_d = nc.dram_tensor(
        "hs", [N_HOP, NSB, 128, NEG * N_MEM], BF16, kind="ExternalInput"
    )
    ts_d = nc.dram_tensor("ts", [BC, N_HOP, N_MEM, DIM], BF16, kind="ExternalInput")
    vs_d = nc.dram_tensor("vs", [BC, DIM], F32, kind="ExternalInput")
    W1_d = nc.dram_tensor("W1", [DIM, D4], F32, kind="ExternalInput")
    b1_d = nc.dram_tensor("b1", [DIM], F32, kind="ExternalInput")
    W2_d = nc.dram_tensor("W2", [1, DIM], F32, kind="ExternalInput")
    Wih_d = nc.dram_tensor("W_ih", [N_HOP, 3 * DIM, DIM], F32, kind="ExternalInput")
    Whh_d = nc.dram_tensor("W_hh", [N_HOP, 3 * DIM, DIM], F32, kind="ExternalInput")
    bih_d = nc.dram_tensor("b_ih", [N_HOP, 3 * DIM], F32, kind="ExternalInput")
    bhh_d = nc.dram_tensor("b_hh", [N_HOP, 3 * DIM], F32, kind="ExternalInput")
    out_d = nc.dram_tensor("out", [BC, DIM], F32, kind="ExternalOutput")

    import contextlib

    with tile.TileContext(nc) as tc:
        with (
            (tc.For_i(0, n_iter, 1) if n_iter > 1 else contextlib.nullcontext()),
            tc.tile_pool(name="consts", bufs=1) as consts,
            tc.tile_pool(name="hop_io", bufs=2) as hop_io,
            tc.tile_pool(name="rpool", bufs=12) as rpool,
            tc.tile_pool(name="ppool", bufs=2) as ppool,
            tc.tile_pool(name="fpool", bufs=2) as fpool,
            tc.tile_pool(name="zpool", bufs=2) as zpool,
            tc.tile_pool(name="apool", bufs=2) as apool,
            tc.tile_pool(name="small", bufs=2) as small,
            tc.tile_pool(name="mstate", bufs=2) as mstate,
            tc.tile_pool(name="pp_rh", bufs=1, space="PSUM") as pp_rh,
            tc.tile_pool(name="pp_zt", bufs=1, space="PSUM") as pp_zt,
            tc.tile_pool(name="pp_m", bufs=2, space="PSUM") as pp_m,
            tc.tile_pool(name="pp_2", bufs=1, space="PSUM") as pp_2,
        ):
            ident = consts.tile([128, 128], F32)
            make_identity(nc, ident)
            ident16 = consts.tile([128, 128], BF16)
            nc.scalar.copy(out=ident16, in_=ident)

            # block-diag ones [p=(q,e4), q]: 1 iff p//4 == q.  PSUM matmul
            # outputs may only start at partition 0/32/64, so super-blocks
            # write 64-row halves: lo has the diag in cols 0-31 (S even),
            # hi in cols 32-63 (S odd); the other half-block's rows get +0.
            a2 = ident.rearrange("p (q two) -> p q two", two=2)
            t64 = consts.tile([128, 64], F32)
            nc.vector.tensor_add(t64, a2[:, :, 0], a2[:, :, 1])
            b2v = t64.rearrange("p (q two) -> p q two", two=2)
            t32 = consts.tile([128, 32], F32)
            nc.vector.tensor_add(t32, b2v[:, :, 0], b2v[:, :, 1])
            ones_lo = consts.tile([128, 64], BF16)
            nc.vector.memset(ones_lo, 0)
            nc.scalar.copy(out=ones_lo[:, 0:32], in_=t32)
            ones_hi = consts.tile([128, 64], BF16)
            nc.vector.memset(ones_hi, 0)
            nc.scalar.copy(out=ones_hi[:, 32:64], in_=t32)

            # ---- weights prep (one-time) ----
            w1_sb = consts.tile([DIM, D4], F32)
            nc.sync.dma_start(out=w1_sb, in_=W1_d[:, :])
            w1t_ps = pp_m.tile([D4, DIM], F32, tag="ps1")
            nc.tensor.transpose(w1t_ps, w1_sb, ident[:DIM, :DIM])
            W1T = consts.tile([D4, DIM], BF16)
            nc.scalar.copy(out=W1T, in_=w1t_ps)

            W2T_f = consts.tile([DIM, 1], F32)
            nc.sync.dma_start(out=W2T_f, in_=W2_d.rearrange("a b -> b a"))
            # block-diag W2^T [128, 4]: col j = W2^T at partitions 32j..
            w2bd_f = consts.tile([128, EV], F32)
            nc.vector.memset(w2bd_f, 0)
            for j in range(EV):
                nc.scalar.copy(
                    out=w2bd_f[j * DIM : (j + 1) * DIM, j : j + 1], in_=W2T_f
                )
            W2BD = consts.tile([128, EV], BF16)
            nc.scalar.copy(out=W2BD, in_=w2bd_f)
            b1T = consts.tile([DIM, 1], F32)
            nc.sync.dma_start(out=b1T, in_=b1_d[:].unsqueeze(1))
            b1T2 = consts.tile([2 * DIM, 1], F32)
            nc.scalar.copy(out=b1T2[0:DIM, :], in_=b1T)
            nc.scalar.copy(out=b1T2[DIM : 2 * DIM, :], in_=b1T)

            WihT, WhhT, bsum_rz, bihn_t, bhhn_t = [], [], [], [], []
            for hop in range(N_HOP):
                wih_sb = consts.tile([3 * DIM, DIM], F32, tag="wload", bufs=4)
                nc.sync.dma_start(out=wih_sb, in_=Wih_d[hop])
                wt_ps = pp_m.tile([DIM, 3 * DIM], F32, tag="ps1")
                nc.tensor.transpose(wt_ps, wih_sb, ident[: 3 * DIM, : 3 * DIM])
                wT = consts.tile([DIM, 3 * DIM], F32, tag=f"wihT{hop}")
                nc.scalar.copy(out=wT, in_=wt_ps)
                WihT.append(wT)

                whh_sb = consts.tile([3 * DIM, DIM], F32, tag="wload", bufs=4)
                nc.sync.dma_start(out=whh_sb, in_=Whh_d[hop])
                wt_ps2 = pp_m.tile([DIM, 3 * DIM], F32, tag="ps1")
                nc.tensor.transpose(wt_ps2, whh_sb, ident[: 3 * DIM, : 3 * DIM])
                wT2 = consts.tile([DIM, 3 * DIM], F32, tag=f"whhT{hop}")
                nc.scalar.copy(out=wT2, in_=wt_ps2)
                WhhT.append(wT2)

                gate_b = []
                for gd, gname in ((bih_d, "ih"), (bhh_d, "hh")):
                    for gate in range(3):
                        bt = consts.tile([DIM, 1], F32, tag=f"b{gname}{hop}{gate}")
                        nc.sync.dma_start(
                            out=bt,
                            in_=gd[hop, gate * DIM : (gate + 1) * DIM].unsqueeze(1),
                        )
                        gate_b.append(bt)
                b_r = consts.tile([DIM, 1], F32, tag=f"b_r{hop}")
                nc.vector.tensor_add(b_r, gate_b[0], gate_b[3])
                b_z = consts.tile([DIM, 1], F32, tag=f"b_z{hop}")
                nc.vector.tensor_add(b_z, gate_b[1], gate_b[4])
                bsum_rz.append((b_r, b_z))
                bihn_t.append(gate_b[2])
                bhhn_t.append(gate_b[5])

            # ---- initial M state ----
            vs_row = consts.tile([BC, DIM], F32)
            nc.sync.dma_start(out=vs_row, in_=vs_d[:, :])
            vst_ps = pp_m.tile([DIM, BC], F32, tag="ps1")
            nc.tensor.transpose(vst_ps, vs_row, ident)
            vsT = consts.tile([DIM, BC], F32)
            nc.scalar.copy(out=vsT, in_=vst_ps)
            MT = vsT  # current M^T [d, b]

            # v_rep [b, (m16, d)] bf16 via packed log-doubling
            v_rep = consts.tile([BC, MC * DIM], BF16)
            nc.vector.tensor_copy(v_rep[:, 0:DIM], vs_row)
            w = DIM
            while w < MC * DIM:
                nc.vector.tensor_copy(v_rep[:, w : 2 * w], v_rep[:, 0:w])
                w *= 2

            M_rep = v_rep  # hop 0: M == vs

            for hop in range(N_HOP):
                # h for the whole hop+sblk: [p=(q,e4), (eg, m)]
                h_sb = []
                for S in range(NSB):
                    h_t = hop_io.tile([128, NEG * N_MEM], BF16, tag="h", bufs=4)
                    nc.scalar.dma_start(out=h_t, in_=hs_d[hop, S])
                    h_sb.append(h_t)
                t_hop = hop_io.tile([BC, N_MEM * DIM], BF16, tag="t_hop")
                nc.scalar.dma_start(
                    out=t_hop, in_=ts_d[:, hop].rearrange("b m d -> b (m d)")
                )

                # Rh accumulators: [b=128, (m16, d32)] f32, one per m-chunk
                rh_ps = [
                    pp_rh.tile([128, MC * DIM], F32, tag=f"rh{c}", name=f"rh{c}")
                    for c in range(NCH)
                ]

                # ---- einsum: Rh = sum_e R*h via PE block-diag reduce ----
                for S in range(NSB):
                    half = (S // 2) * 64
                    ones_bd = ones_lo if S % 2 == 0 else ones_hi
                    for quad in range(NEG // 4):
                        P_ts = []
                        A01 = A23 = None
                        for e4i in range(4):
                            eg = quad * 4 + e4i
                            r_t = rpool.tile(
                                [128, DIM * N_MEM], BF16, tag="R", bufs=16
                            )
                            nc.sync.dma_start(out=r_t, in_=Rs_d[hop, S, eg])
                            if variant == "dma":
                                continue
                            # single DVE broadcast-mul (h stride-0 over d,
                            # packed inner m): ~750 ns measured on HW
                            P_t = ppool.tile(
                                [128, DIM * N_MEM], BF16, tag="P", bufs=8
                            )
                            h_sl = h_sb[S][
                                :, eg * N_MEM : (eg + 1) * N_MEM
                            ]
                            h_b = h_sl.unsqueeze(1).broadcast_to(
                                (128, DIM, N_MEM)
                            )
                            nc.vector.tensor_tensor(
                                P_t.rearrange("p (d m) -> p d m", d=DIM),
                                r_t.rearrange("p (d m) -> p d m", d=DIM),
                                h_b,
                                op=ALU.mult,
                            )
                            P_ts.append(P_t)
                            if variant == "nomm":
                                continue
                            # quad-sum tree on DVE, emitted as operands land
                            if e4i == 1:
                                A01 = ppool.tile(
                                    [128, DIM * N_MEM], BF16, tag="PA", bufs=3
                                )
                                nc.vector.tensor_add(A01, P_ts[0], P_ts[1])
                            elif e4i == 3:
                                A23 = ppool.tile(
                                    [128, DIM * N_MEM], BF16, tag="PB", bufs=3
                                )
                                nc.vector.tensor_add(A23, P_ts[2], P_ts[3])
                        if variant in ("dma", "nomm"):
                            continue
                        AQ = ppool.tile([128, DIM * N_MEM], BF16, tag="PQ",
                                        bufs=3)
                        nc.vector.tensor_add(AQ, A01, A23)
                        Pm = AQ.rearrange("p (d m) -> p m d", d=DIM)
                        for c in range(NCH):
                            nc.tensor.matmul(
                                rh_ps[c][half : half + 64, :],
                                lhsT=ones_bd,
                                rhs=Pm[:, c * MC : (c + 1) * MC, :],
                                start=(S % 2 == 0 and quad == 0),
                                stop=(S % 2 == 1 and quad == NEG // 4 - 1),
                            )

                if variant in ("dma", "nomm"):
                    continue
                if variant == "notail":
                    # consume rh into out cheaply to keep deps
                    if hop == N_HOP - 1:
                        M_row = mstate.tile([BC, DIM], F32, tag="M_row")
                        nc.scalar.copy(out=M_row, in_=rh_ps[0][:, 0:DIM])
                        nc.sync.dma_start(out=out_d[:, :], in_=M_row)
                    else:
                        for c in range(NCH):
                            rh_sb = fpool.tile([BC, MC * DIM], BF16, tag="rh_sb")
                            nc.scalar.copy(out=rh_sb, in_=rh_ps[c])
                    continue

                # ---- features z = [Rh*v, Rh*M, |Rh-v|, |Rh-M|] ----
                z_hop = zpool.tile([BC, N_MEM * 4 * DIM], BF16, tag="z")
                z4 = z_hop.rearrange("b (m f d) -> b m f d", f=4, d=DIM)
                vr3 = v_rep.rearrange("b (m d) -> b m d", d=DIM)
                mr3 = M_rep.rearrange("b (m d) -> b m d", d=DIM)
                for c in range(NCH):
                    mc = slice(c * MC, (c + 1) * MC)
                    rh_sb = fpool.tile([BC, MC * DIM], BF16, tag="rh_sb")
                    if c % 2 == 0:
                        nc.scalar.copy(out=rh_sb, in_=rh_ps[c])
                    else:
                        nc.vector.tensor_copy(rh_sb, rh_ps[c])
                    rh3 = rh_sb.rearrange("b (m d) -> b m d", d=DIM)
                    nc.vector.tensor_mul(z4[:, mc, 0, :], rh3, vr3)
                    nc.vector.tensor_mul(z4[:, mc, 1, :], rh3, mr3)
                    nc.gpsimd.tensor_tensor(
                        z4[:, mc, 2, :], rh3, vr3, op=ALU.subtract
                    )
                    nc.gpsimd.tensor_tensor(
                        z4[:, mc, 3, :], rh3, mr3, op=ALU.subtract
                    )
                    nc.scalar.activation(
                        z4[:, mc, 2:4, :], z4[:, mc, 2:4, :], AF.Abs
                    )

                # ---- MLP per m4-group: transpose + matmuls; groups of 4
                # stack a1 into [128, 512] (32-aligned partition offsets) so
                # one block-diag W2 matmul emits Z for 4 groups at once.
                # Final Z col-order is m' = (m4, G, g4); ts is host-permuted
                # to match (softmax is order-invariant).
                zT_ps = pp_m.tile([BC, N_MEM], F32, tag="ps1")
                zf = z_hop.rearrange("b (m fd) -> b m fd", fd=4 * DIM)
                for G in range(EV):
                    a1_4 = apool.tile([128, EV * BC], BF16, tag="a1")
                    for g4 in range(EV):
                        g = G * EV + g4
                        zt_ps = pp_zt.tile([D4, EV * BC], BF16, tag="zt")
                        for j in range(EV):
                            nc.tensor.transpose(
                                zt_ps[:, j * BC : (j + 1) * BC],
                                zf[:, g * EV + j, :],
                                ident16,
                            )
                        zt_sb = zpool.tile(
                            [D4, EV * BC], BF16, tag="zt_sb", bufs=3
                        )
                        if g % 2 == 0:
                            nc.scalar.copy(out=zt_sb, in_=zt_ps)
                        else:
                            nc.vector.tensor_copy(zt_sb, zt_ps)
                        ps1 = pp_m.tile([DIM, EV * BC], F32, tag="ps1")
                        nc.tensor.matmul(
                            ps1, lhsT=W1T, rhs=zt_sb, start=True, stop=True
                        )
                        nc.scalar.activation(
                            a1_4[g4 * DIM : (g4 + 1) * DIM, :],
                            ps1,
                            AF.Tanh,
                            bias=b1T,
                        )
                    ps2 = pp_2.tile([EV, EV * BC], F32, tag="ps2")
                    nc.tensor.matmul(ps2, lhsT=W2BD, rhs=a1_4, start=True, stop=True)
                    z4sb = zpool.tile([EV, EV * BC], F32, tag="z4sb", bufs=2)
                    if G % 2 == 0:
                        nc.scalar.copy(out=z4sb, in_=ps2)
                    else:
                        nc.vector.tensor_copy(z4sb, ps2)
                    for j in range(EV):
                        nc.tensor.transpose(
                            zT_ps[:, j * MC + G * EV : j * MC + (G + 1) * EV],
                            z4sb[:, j * BC : (j + 1) * BC],
                            ident[:EV, :EV],
                        )

                # softmax over m (skip max-subtract; |Z| tanh-bounded),
                # normalize o after the t-reduction
                e_row = small.tile([BC, N_MEM], F32, tag="e_row")
                nc.scalar.activation(e_row, zT_ps, AF.Exp)
                e16 = small.tile([BC, N_MEM], BF16, tag="e16")
                nc.scalar.copy(out=e16, in_=e_row)
                ssum = small.tile([BC, 1], F32, tag="ssum")
                nc.vector.tensor_reduce(out=ssum, in_=e_row, axis=AX.X, op=ALU.add)
                rsum = small.tile([BC, 1], F32, tag="rsum")
                nc.vector.reciprocal(rsum, ssum)

                # o[b,d] = (sum_m t[b,m,d] * e[b,m]) * rsum[b]
                t3 = t_hop.rearrange("b (m d) -> b m d", d=DIM)
                g3 = e16.unsqueeze(2).broadcast_to((BC, N_MEM, DIM))
                nc.vector.tensor_tensor(t3, t3, g3, op=ALU.mult)
                for mh in (32, 16, 8, 4, 2):
                    nc.vector.tensor_add(
                        t3[:, :mh, :], t3[:, :mh, :], t3[:, mh : 2 * mh, :]
                    )
                o_raw = small.tile([BC, DIM], F32, tag="o_raw")
                nc.vector.tensor_add(
                    o_raw.unsqueeze(1), t3[:, 0:1, :], t3[:, 1:2, :]
                )
                o_row = small.tile([BC, DIM], F32, tag="o_row")
                nc.vector.tensor_scalar_mul(o_row, o_raw, rsum)

                # ---- GRU (transposed layout [*, b], f32) ----
                ot_ps = pp_m.tile([DIM, BC], F32, tag="ps1")
                nc.tensor.transpose(ot_ps, o_row, ident)
                oT = small.tile([DIM, BC], F32, tag="oT")
                nc.scalar.copy(out=oT, in_=ot_ps)

                def gate_pair(g):
                    gi = pp_m.tile([DIM, BC], F32, tag="ps1")
                    nc.tensor.matmul(
                        gi,
                        lhsT=WihT[hop][:, g * DIM : (g + 1) * DIM],
                        rhs=oT,
                        start=True,
                        stop=True,
                    )
                    gh = pp_m.tile([DIM, BC], F32, tag="ps1")
                    nc.tensor.matmul(
                        gh,
                        lhsT=WhhT[hop][:, g * DIM : (g + 1) * DIM],
                        rhs=MT,
                        start=True,
                        stop=True,
                    )
                    return gi, gh

                rz_t = []
                for g in range(2):
                    gi, gh = gate_pair(g)
                    gh_sb = small.tile([DIM, BC], F32, tag=f"gh{g}sb")
                    nc.vector.tensor_copy(gh_sb, gh)
                    gb = small.tile([DIM, BC], F32, tag=f"g{g}b")
                    # (gi + b) + gh in one DVE op (only one PSUM operand)
                    nc.vector.scalar_tensor_tensor(
                        out=gb, in0=gi, scalar=bsum_rz[hop][g], in1=gh_sb,
                        op0=ALU.add, op1=ALU.add,
                    )
                    gt = small.tile([DIM, BC], F32, tag=f"gate{g}")
                    nc.scalar.activation(gt, gb, AF.Sigmoid)
                    rz_t.append(gt)
                r_t, z_t = rz_t

                gi_n, gh_n = gate_pair(2)
                n1 = small.tile([DIM, BC], F32, tag="n1")
                # (gh_n + b_hh_n) * r
                nc.vector.scalar_tensor_tensor(
                    out=n1, in0=gh_n, scalar=bhhn_t[hop], in1=r_t,
                    op0=ALU.add, op1=ALU.mult,
                )
                n2 = small.tile([DIM, BC], F32, tag="n2")
                # (gi_n + b_ih_n) + n1
                nc.vector.scalar_tensor_tensor(
                    out=n2, in0=gi_n, scalar=bihn_t[hop], in1=n1,
                    op0=ALU.add, op1=ALU.add,
                )
                n_t = small.tile([DIM, BC], F32, tag="n_t")
                nc.scalar.activation(n_t, n2, AF.Tanh)

                # M' = n + z * (M - n)
                MT_new = mstate.tile([DIM, BC], F32, tag="MT")
                nc.vector.tensor_sub(MT_new, MT, n_t)
                nc.vector.tensor_mul(MT_new, MT_new, z_t)
                nc.vector.tensor_add(MT_new, MT_new, n_t)
                MT = MT_new

                mrow_ps = pp_m.tile([BC, DIM], F32, tag="ps1")
                nc.tensor.transpose(mrow_ps, MT, ident[:DIM, :DIM])
                if hop < N_HOP - 1:
                    # M_rep [b, (m16, d)] bf16 via packed log-doubling
                    M_rep_new = mstate.tile([BC, MC * DIM], BF16, tag="M_rep")
                    nc.scalar.copy(out=M_rep_new[:, 0:DIM], in_=mrow_ps)
                    w = DIM
                    while w < MC * DIM:
                        nc.vector.tensor_copy(
                            M_rep_new[:, w : 2 * w], M_rep_new[:, 0:w]
                        )
                        w *= 2
                    M_rep = M_rep_new
                else:
                    M_row = mstate.tile([BC, DIM], F32, tag="M_row")
                    nc.scalar.copy(out=M_row, in_=mrow_ps)
                    nc.sync.dma_start(out=out_d[:, :], in_=M_row)

    nc.compile()
    return nc


_NC_CACHE = None


def _get_nc():
    global _NC_CACHE
    if _NC_CACHE is None:
        _NC_CACHE = build_nc()
    return _NC_CACHE


def _bf16(x):
    import ml_dtypes

    return np.asarray(x).astype(ml_dtypes.bfloat16)


def _fp8(x):
    import ml_dtypes

    return np.asarray(x).astype(ml_dtypes.float8_e4m3)


def permute_R(x):
    """Rs [BC, N_HOP, m, d, e] -> [hop, S, eg, (q,e4), (d,m)] bf16."""
    y = x.reshape(NSB, QB, N_HOP, N_MEM, DIM, NEG, EV)
    # [S, q, hop, m, d, eg, e4] -> [hop, S, eg, q, e4, d, m]
    y = y.transpose(2, 0, 5, 1, 6, 4, 3).reshape(
        N_HOP, NSB, NEG, 128, DIM * N_MEM
    )
    return np.ascontiguousarray(_bf16(y))


def permute_h(x):
    """hs [BC, N_HOP, m, e] -> [hop, S, (q,e4), (eg,m)] bf16."""
    y = x.reshape(NSB, QB, N_HOP, N_MEM, NEG, EV)
    # [S, q, hop, m, eg, e4] -> [hop, S, q, e4, eg, m]
    y = y.transpose(2, 0, 1, 5, 4, 3)
    return np.ascontiguousarray(y.reshape(N_HOP, NSB, 128, NEG * N_MEM))


def permute_t(x):
    """ts [BC, hop, m, d]: m reordered to m' = (m4, G, g4), m = (4G+g4)*4+m4."""
    y = x.reshape(BC, N_HOP, EV, EV, EV, DIM)  # [b, hop, G, g4, m4, d]
    return np.ascontiguousarray(
        y.transpose(0, 1, 4, 2, 3, 5).reshape(BC, N_HOP, N_MEM, DIM)
    )


def make_in_maps(hs, Rs, ts, vs, W1, b1, W2, W_ih, W_hh, b_ih, b_hh):
    in_maps = []
    for c in range(N_CORES):
        sl = slice(c * BC, (c + 1) * BC)
        in_maps.append(
            {
                "Rs": permute_R(Rs[sl]),
                "hs": permute_h(_bf16(hs[sl])),
                "ts": permute_t(_bf16(ts[sl])),
                "vs": np.ascontiguousarray(vs[sl]),
                "W1": np.ascontiguousarray(W1),
                "b1": np.ascontiguousarray(b1),
                "W2": np.ascontiguousarray(W2),
                "W_ih": np.ascontiguousarray(W_ih),
                "W_hh": np.ascontiguousarray(W_hh),
                "b_ih": np.ascontiguousarray(b_ih),
                "b_hh": np.ascontiguousarray(b_hh),
            }
        )
    return in_maps


def kernel(hs, Rs, ts, vs, W1, b1, W2, b2, W_ih, W_hh, b_ih, b_hh):
    from concourse.bass_utils import run_bass_kernel_spmd

    nc = _get_nc()
    in_maps = make_in_maps(hs, Rs, ts, vs, W1, b1, W2, W_ih, W_hh, b_ih, b_hh)
    res = run_bass_kernel_spmd(nc, in_maps, list(range(N_CORES)))
    return np.concatenate([r["out"] for r in res.results], axis=0)
